# revision 15
# baseline (speedup 1.0000x reference)
"""Trainium2 Bass kernel for nn_NeighborhoodAttention (GNN message passing).

Strategy (no collectives needed):
  - Host: sort edges by dst, pad nodes 50000->50176 = 392 tiles of 128; core c
    owns 49 contiguous node tiles and their (contiguous) edges. Per node tile,
    the edge list is padded to a multiple of 128 ("blocks"); within each core,
    tiles are processed in descending-edge-count order so the per-position
    block count B_j is shared across all 8 cores (single SPMD program).
  - Gathered endpoint features are staged feature-major ([feat, edge]); the
    per-edge MLPs run feature-major on TensorE with stationary weights.
    Residual layers and the q-dot are folded on the host:
      scores = ((I+kW1) @ qmask)^T h_k        (per-head additive consts cancel
      v      = (I+vW1)^T h_v + vb1             exactly in the segment softmax,
                                               incl. the max subtraction)
  - Edge-major tensors (scores_em, v_em) come from "em-mode" matmuls using the
    h-slab slice as the stationary operand - no transposes anywhere.
  - Segment softmax/scatter-add: per 128-edge block build onehot[e, n] on DVE
    (iota + is_equal vs dst_local), then accumulate in PSUM per node tile:
      S[:, 0:128] += onehot^T @ (v_em * exp(scores)_bcast),
      S[:, 128:136] += onehot^T @ exp(scores).
  - Node side: aggr = S / max(S1,eps) per head, relu, PE-transpose to
    feature-major, 2-layer output MLP, write feature-major; host untransposes.
"""
import os
import sys
from contextlib import ExitStack

import numpy as np

sys.path.insert(0, "/opt/trn_rl_repo")

import concourse.bass as bass
import concourse.tile as tile
from concourse import mybir
from concourse.bass_utils import run_bass_kernel_spmd
from concourse.vector_clock import ScopedClock


def _patched_drain_and_barrier(self, tick_clock, wait_clock):
    # Workaround: walrus CoreV3 setupSyncWait rejects >couple sem-waits on a
    # CTRL-class (drain) instruction. Spread the tail-drain waits across
    # preceding sync-engine nops (1 wait each) and leave the drain clean.
    nc = self.nc
    nop0 = nc.sync.nop(hint="tile_drain_waits", nofuse=True)
    wait_clock.add_sem_waits(nop0.ins, ScopedClock({None: tick_clock.global_clock}))
    si = nop0.ins.sync_info
    waits = list(si.on_wait) if si is not None and si.on_wait else []
    if len(waits) > 1:
        si.on_wait = waits[:1]
        for w in waits[1:]:
            ni = nc.sync.nop(hint="tile_drain_waits", nofuse=True)
            nsi = ni.ins.sync_info
            if nsi is None:
                ni.ins.sync_info = mybir.SyncInfo(on_wait=[w], on_update=[])
            else:
                nsi.on_wait = [w]
    nc.sync.drain()
    nc.all_engine_barrier()
    popped = nc._tile_sem_poison_stack.pop()
    assert popped is self._sem_poison
    nc.clear_and_free_semaphores(list(self.sems.allocated().values()))
    nc.all_engine_barrier()


tile.TileContext._drain_and_barrier = _patched_drain_and_barrier


def _split_excess_waits(nc, max_waits=1):
    """Walrus CoreV3 setupSyncWait rejects instructions with more than one
    sem-wait. Hoist excess waits onto same-engine nops inserted just before
    the offending instruction (program order per engine is the bb order)."""
    f = nc.m.functions[0]
    offenders = {}  # name -> list of hoisted-nop Instructions
    created = set()
    for bb in f.blocks:
        for inst in bb.instructions:
            si = inst.sync_info
            if si is None or not si.on_wait or len(si.on_wait) <= max_waits:
                continue
            w = list(si.on_wait)
            nops = []
            for wt in w[:-max_waits]:
                bi = nc.engines[inst.engine].nop(nofuse=True)
                nsi = bi.ins.sync_info
                if nsi is None:
                    bi.ins.sync_info = mybir.SyncInfo(on_wait=[wt], on_update=[])
                else:
                    nsi.on_wait = [wt]
                nops.append(bi.ins)
                created.add(bi.ins.name)
            si.on_wait = w[-max_waits:]
            offenders[inst.name] = nops
    if not offenders:
        return
    for bb in f.blocks:
        insts = list(bb.instructions)
        out = []
        changed = False
        for inst in insts:
            if inst.name in created:
                changed = True
                continue
            if inst.name in offenders:
                out.extend(offenders[inst.name])
                changed = True
            out.append(inst)
        if changed:
            bb.instructions = out

# problem constants (hardcoded per contract)
N, E = 50000, 800000
SRCF, DSTF, EDGEF = 64, 64, 32
D, H, DH = 128, 8, 16
SCALE = 1.0 / np.sqrt(np.float32(DH))
NCORES = 8
P = 128
NT_TOTAL = 392
TPC = NT_TOTAL // NCORES        # 49 node tiles per core
NPC = TPC * P                   # 6272 nodes per core
SLAB_BLOCKS = 16                # edge DMA/L0 slab = 16 blocks = 2048 edges
F32 = mybir.dt.float32
I32 = mybir.dt.int32


# ----------------------------------------------------------------- host prep
def _prep(inputs):
    x_src = np.asarray(inputs["x_src"], np.float32)
    x_dst = np.asarray(inputs["x_dst"], np.float32)
    edge_attr = np.asarray(inputs["edge_attr"], np.float32)
    ei = np.asarray(inputs["edge_index"])
    src = ei[0].astype(np.int64)
    dst = ei[1].astype(np.int64)

    perm = np.argsort(dst, kind="stable")
    src_s, dst_s = src[perm], dst[perm]
    ea_s = edge_attr[perm]
    tile_counts = np.bincount(dst_s // P, minlength=NT_TOTAL)
    tile_starts = np.zeros(NT_TOTAL + 1, np.int64)
    np.cumsum(tile_counts, out=tile_starts[1:])

    orders = np.zeros((NCORES, TPC), np.int64)
    sorted_counts = np.zeros((NCORES, TPC), np.int64)
    for c in range(NCORES):
        tiles = np.arange(c * TPC, (c + 1) * TPC)
        o = np.argsort(-tile_counts[tiles], kind="stable")
        orders[c] = tiles[o]
        sorted_counts[c] = tile_counts[orders[c]]
    B = np.maximum(np.ceil(sorted_counts.max(axis=0) / P).astype(np.int64), 1)
    Bcum = np.zeros(TPC + 1, np.int64)
    np.cumsum(B, out=Bcum[1:])
    NBLK = int(B.sum())
    EPAD = NBLK * P

    slot = np.full((NCORES, EPAD), -1, np.int64)
    dstloc = np.full((NCORES, EPAD), -1, np.int64)
    for c in range(NCORES):
        for j in range(TPC):
            t = orders[c, j]
            s0, cnt = int(tile_starts[t]), int(tile_counts[t])
            pos = int(Bcum[j]) * P
            slot[c, pos:pos + cnt] = np.arange(s0, s0 + cnt)
            dstloc[c, pos:pos + cnt] = dst_s[s0:s0 + cnt] - t * P

    real = slot >= 0
    slot_c = np.where(real, slot, 0)
    XA = np.zeros((NCORES, 128, EPAD), np.float32)
    XB = np.zeros((NCORES, 33, EPAD), np.float32)
    for c in range(NCORES):
        r = real[c]
        XA[c, :64] = np.where(r, x_src[src_s[slot_c[c]]].T, 0)
        XA[c, 64:] = np.where(r, x_dst[dst_s[slot_c[c]]].T, 0)
        XB[c, :32] = np.where(r, ea_s[slot_c[c]].T, 0)
        XB[c, 32] = 1.0
    dstlocT = np.ascontiguousarray(
        dstloc.reshape(NCORES, NBLK, P).transpose(0, 2, 1)).astype(np.float32)

    kW0 = np.asarray(inputs["kW0"], np.float32)
    kb0 = np.asarray(inputs["kb0"], np.float32)
    kW1 = np.asarray(inputs["kW1"], np.float32)
    vW0 = np.asarray(inputs["vW0"], np.float32)
    vb0 = np.asarray(inputs["vb0"], np.float32)
    vW1 = np.asarray(inputs["vW1"], np.float32)
    vb1 = np.asarray(inputs["vb1"], np.float32)
    q = np.asarray(inputs["q"], np.float32)

    qmask = np.zeros((D, H), np.float32)
    for h in range(H):
        qmask[h * DH:(h + 1) * DH, h] = q[0, h * DH:(h + 1) * DH] * SCALE

    weights = dict(
        W0kA=np.ascontiguousarray(kW0[:128]),
        W0kB=np.concatenate([kW0[128:160], kb0[None, :]], 0),
        W0vA=np.ascontiguousarray(vW0[:128]),
        W0vB=np.concatenate([vW0[128:160], vb0[None, :]], 0),
        AQ8=(np.eye(D, dtype=np.float32) + kW1) @ qmask,
        MW1v=np.eye(D, dtype=np.float32) + vW1,
        b1v_rep=np.tile(vb1[None, :], (P, 1)),
        oW0=np.asarray(inputs["oW0"], np.float32),
        ob0=np.asarray(inputs["ob0"], np.float32).reshape(P, 1),
        MoW1=np.eye(D, dtype=np.float32) + np.asarray(inputs["oW1"], np.float32),
        ob1=np.asarray(inputs["ob1"], np.float32).reshape(P, 1),
    )
    use_b1v = bool(np.any(weights["b1v_rep"]))
    meta = dict(B=B, Bcum=Bcum, NBLK=NBLK, EPAD=EPAD, orders=orders,
                use_b1v=use_b1v)
    staged = dict(XA=XA, XB=XB, dstlocT=dstlocT)
    return staged, weights, meta


def _unshard(out_cores, orders):
    full = np.zeros((NT_TOTAL * P, D), np.float32)
    for c in range(NCORES):
        for j in range(TPC):
            t = int(orders[c, j])
            full[t * P:(t + 1) * P] = out_cores[c][:, j * P:(j + 1) * P].T
    return np.ascontiguousarray(full[:N])


# ------------------------------------------------------------- bass program
def build_program(B, Bcum, NBLK, EPAD, use_b1v, tpc=TPC, npc=None):
    npc = npc if npc is not None else tpc * P
    nc = bass.Bass("TRN2", target_bir_lowering=False, debug=False)
    XA_d = nc.declare_dram_parameter("XA", [128, EPAD], F32, isOutput=False)
    XB_d = nc.declare_dram_parameter("XB", [33, EPAD], F32, isOutput=False)
    DL_d = nc.declare_dram_parameter("DL", [128, NBLK], F32, isOutput=False)
    wnames = ["W0kA", "W0kB", "W0vA", "W0vB", "AQ8", "MW1v", "b1v_rep",
              "oW0", "ob0", "MoW1", "ob1"]
    wshapes = {"W0kA": [128, 128], "W0kB": [33, 128], "W0vA": [128, 128],
               "W0vB": [33, 128], "AQ8": [128, 8], "MW1v": [128, 128],
               "b1v_rep": [128, 128], "oW0": [128, 128], "ob0": [128, 1],
               "MoW1": [128, 128], "ob1": [128, 1]}
    w_d = {n: nc.declare_dram_parameter(n, wshapes[n], F32, isOutput=False)
           for n in wnames}
    OUT_d = nc.declare_dram_parameter("OUT", [128, npc], F32, isOutput=True)

    SLAB = SLAB_BLOCKS * P
    nslabs = (NBLK + SLAB_BLOCKS - 1) // SLAB_BLOCKS

    with ExitStack() as ctx:
        tc = ctx.enter_context(tile.TileContext(nc))
        cpool = ctx.enter_context(tc.tile_pool(name="consts", bufs=1))
        xpool = ctx.enter_context(tc.tile_pool(name="x", bufs=3))
        hpool = ctx.enter_context(tc.tile_pool(name="h", bufs=2))
        empool = ctx.enter_context(tc.tile_pool(name="em", bufs=3))
        npool = ctx.enter_context(tc.tile_pool(name="node", bufs=2))
        ps_l0 = ctx.enter_context(tc.tile_pool(name="psl0", bufs=1, space="PSUM"))
        ps_em = ctx.enter_context(tc.tile_pool(name="psem", bufs=2, space="PSUM"))
        ps_s = ctx.enter_context(tc.tile_pool(name="pss", bufs=1, space="PSUM"))
        ps_n = ctx.enter_context(tc.tile_pool(name="psn", bufs=1, space="PSUM"))

        # --- persistent constants ---
        w_sb = {}
        for n in wnames:
            t = cpool.tile(wshapes[n], F32, name=f"w_{n}")
            nc.sync.dma_start(t[:], w_d[n][:])
            w_sb[n] = t
        dl_sb = cpool.tile([128, NBLK], F32, name="dl")
        nc.sync.dma_start(dl_sb[:], DL_d[:])
        iota_row_i = cpool.tile([128, 128], I32, name="iota_row_i")
        nc.gpsimd.iota(iota_row_i[:], pattern=[[1, 128]], base=0,
                       channel_multiplier=0)
        iota_row = cpool.tile([128, 128], F32, name="iota_row")
        nc.vector.tensor_copy(iota_row[:], iota_row_i[:])
        iota_p_i = cpool.tile([128, 1], I32, name="iota_p_i")
        nc.gpsimd.iota(iota_p_i[:], pattern=[[1, 1]], base=0,
                       channel_multiplier=1)
        iota_p = cpool.tile([128, 1], F32, name="iota_p")
        nc.vector.tensor_copy(iota_p[:], iota_p_i[:])
        ident = cpool.tile([128, 128], F32, name="ident")
        nc.vector.tensor_scalar(ident[:], iota_row[:], iota_p[:], None,
                                op0=mybir.AluOpType.is_equal)

        headsel = np.arange(D) // DH  # feature -> head

        # --- main loop ---
        xa_t = xb_t = hk_t = hv_t = None
        slab_edges = 0
        j = 0  # current node tile
        S_ps = None
        for s in range(nslabs):
            b0 = s * SLAB_BLOCKS
            nblk_s = min(SLAB_BLOCKS, NBLK - b0)
            ne = nblk_s * P
            e0 = b0 * P
            xa_t = xpool.tile([128, SLAB], F32, tag="xa", name=f"xa{s}")
            xb_t = xpool.tile([33, SLAB], F32, tag="xb", name=f"xb{s}")
            nc.sync.dma_start(xa_t[:, :ne], XA_d[:, e0:e0 + ne])
            nc.sync.dma_start(xb_t[:, :ne], XB_d[:, e0:e0 + ne])
            hk_t = hpool.tile([128, SLAB], F32, tag="hk", name=f"hk{s}")
            hv_t = hpool.tile([128, SLAB], F32, tag="hv", name=f"hv{s}")
            # L0 in chunks of 512
            for c0 in range(0, ne, 512):
                cw = min(512, ne - c0)
                hk_ps = ps_l0.tile([128, 512], F32, tag="hkps", name=f"hkps{s}_{c0}")
                nc.tensor.matmul(hk_ps[:, :cw], w_sb["W0kA"][:],
                                 xa_t[:, c0:c0 + cw], start=True, stop=False)
                nc.tensor.matmul(hk_ps[:, :cw], w_sb["W0kB"][:],
                                 xb_t[:, c0:c0 + cw], start=False, stop=True)
                nc.scalar.activation(hk_t[:, c0:c0 + cw], hk_ps[:, :cw],
                                     mybir.ActivationFunctionType.Relu)
                hv_ps = ps_l0.tile([128, 512], F32, tag="hvps", name=f"hvps{s}_{c0}")
                nc.tensor.matmul(hv_ps[:, :cw], w_sb["W0vA"][:],
                                 xa_t[:, c0:c0 + cw], start=True, stop=False)
                nc.tensor.matmul(hv_ps[:, :cw], w_sb["W0vB"][:],
                                 xb_t[:, c0:c0 + cw], start=False, stop=True)
                nc.vector.tensor_scalar_max(hv_t[:, c0:c0 + cw], hv_ps[:, :cw],
                                            0.0)
            # em + scatter per block
            for bb in range(nblk_s):
                b = b0 + bb
                if b == Bcum[j]:
                    S_ps = ps_s.tile([128, 128], F32, tag="S", name=f"S{j}")
                    S1_ps = ps_s.tile([128, 8], F32, tag="S1", name=f"S1_{j}")
                first = (b == Bcum[j])
                last = (b == Bcum[j + 1] - 1)
                sl = slice(bb * P, (bb + 1) * P)

                vs_ps = ps_em.tile([128, 144], F32, tag="vs", name=f"vs{b}")
                nc.tensor.matmul(vs_ps[:, 128:136], hk_t[:, sl], w_sb["AQ8"][:],
                                 start=True, stop=True, skip_group_check=True)
                nc.tensor.matmul(vs_ps[:, 0:128], hv_t[:, sl], w_sb["MW1v"][:],
                                 start=True, stop=True, skip_group_check=True)
                ex8 = empool.tile([128, 8], F32, tag="ex8", name=f"ex8_{b}")
                nc.scalar.activation(ex8[:], vs_ps[:, 128:136],
                                     mybir.ActivationFunctionType.Exp)
                if use_b1v:
                    vtmp = empool.tile([128, 128], F32, tag="vtmp", name=f"vt{b}")
                    nc.vector.tensor_add(vtmp[:], vs_ps[:, 0:128],
                                         w_sb["b1v_rep"][:])
                    vsrc = vtmp
                else:
                    vsrc = vs_ps
                exv = empool.tile([128, 128], F32, tag="exv", name=f"exv{b}")
                vap = (vsrc[:, 0:128] if not use_b1v else vsrc[:])
                nc.vector.tensor_tensor(
                    exv[:].rearrange("p (h r) -> p h r", r=DH),
                    vap.rearrange("p (h r) -> p h r", r=DH),
                    ex8[:].unsqueeze(2).broadcast_to([128, 8, DH]),
                    op=mybir.AluOpType.mult)
                oh = empool.tile([128, 128], F32, tag="oh", name=f"oh{b}")
                nc.vector.tensor_scalar(oh[:], iota_row[:], dl_sb[:, b:b + 1],
                                        None, op0=mybir.AluOpType.is_equal)
                nc.tensor.matmul(S_ps[:], oh[:], exv[:],
                                 start=first, stop=last, skip_group_check=True)
                nc.tensor.matmul(S1_ps[:], oh[:], ex8[:],
                                 start=first, stop=last, skip_group_check=True)

                if last:
                    # node-tile epilogue
                    s1 = npool.tile([128, 8], F32, tag="s1", name=f"s1_{j}")
                    nc.vector.tensor_scalar_max(s1[:], S1_ps[:], 1e-30)
                    r1 = npool.tile([128, 8], F32, tag="r1", name=f"r1_{j}")
                    nc.vector.reciprocal(r1[:], s1[:])
                    g = npool.tile([128, 128], F32, tag="g", name=f"g{j}")
                    nc.vector.tensor_tensor(
                        g[:].rearrange("p (h r) -> p h r", r=DH),
                        S_ps[:].rearrange("p (h r) -> p h r", r=DH),
                        r1[:].unsqueeze(2).broadcast_to([128, 8, DH]),
                        op=mybir.AluOpType.mult)
                    nc.vector.tensor_scalar_max(g[:], g[:], 0.0)
                    tp_ps = ps_n.tile([128, 128], F32, tag="nps", name=f"tp{j}")
                    nc.tensor.transpose(tp_ps[:], g[:], ident[:])
                    gfm = npool.tile([128, 128], F32, tag="gfm", name=f"gfm{j}")
                    nc.scalar.copy(gfm[:], tp_ps[:])
                    h0_ps = ps_n.tile([128, 128], F32, tag="nps", name=f"h0p{j}")
                    nc.tensor.matmul(h0_ps[:], w_sb["oW0"][:], gfm[:],
                                     start=True, stop=True)
                    h0 = npool.tile([128, 128], F32, tag="h0", name=f"h0{j}")
                    nc.scalar.activation(h0[:], h0_ps[:],
                                         mybir.ActivationFunctionType.Relu,
                                         bias=w_sb["ob0"][:])
                    o2_ps = ps_n.tile([128, 128], F32, tag="nps", name=f"o2p{j}")
                    nc.tensor.matmul(o2_ps[:], w_sb["MoW1"][:], h0[:],
                                     start=True, stop=True)
                    ot = npool.tile([128, 128], F32, tag="ot", name=f"ot{j}")
                    nc.scalar.activation(ot[:], o2_ps[:],
                                         mybir.ActivationFunctionType.Relu,
                                         bias=w_sb["ob1"][:])
                    nc.sync.dma_start(OUT_d[:, j * P:(j + 1) * P], ot[:])
                    j += 1
    _split_excess_waits(nc)
    return nc


# ------------------------------------------------------------------ kernel
def kernel(**inputs):
    staged, weights, meta = _prep(inputs)
    nc = build_program(meta["B"], meta["Bcum"], meta["NBLK"], meta["EPAD"],
                       meta["use_b1v"])
    in_maps = []
    for c in range(NCORES):
        m = {"XA": staged["XA"][c], "XB": staged["XB"][c],
             "DL": staged["dstlocT"][c]}
        m.update(weights)
        in_maps.append(m)
    res = run_bass_kernel_spmd(nc, in_maps, list(range(NCORES)))
    out_cores = [res.results[c]["OUT"] for c in range(NCORES)]
    return _unshard(out_cores, meta["orders"])


# revision 16
# speedup vs baseline: 2.4027x; 2.4027x over previous
"""Trainium2 Bass kernel for nn_NeighborhoodAttention (GNN message passing).

Strategy (no collectives needed):
  - Host: sort edges by dst, pad nodes 50000->50176 = 392 tiles of 128; core c
    owns 49 contiguous node tiles and their (contiguous) edges. Per node tile,
    the edge list is padded to a multiple of 128 ("blocks"); within each core,
    tiles are processed in descending-edge-count order so the per-position
    block count B_j is shared across all 8 cores (single SPMD program).
  - Gathered endpoint features are staged feature-major ([feat, edge]); the
    per-edge MLPs run feature-major on TensorE with stationary weights.
    Residual layers and the q-dot are folded on the host:
      scores = ((I+kW1) @ qmask)^T h_k        (per-head additive consts cancel
      v      = (I+vW1)^T h_v + vb1             exactly in the segment softmax,
                                               incl. the max subtraction)
  - Edge-major tensors (scores_em, v_em) come from "em-mode" matmuls using the
    h-slab slice as the stationary operand - no transposes anywhere.
  - Segment softmax/scatter-add: per 128-edge block build onehot[e, n] on DVE
    (iota + is_equal vs dst_local), then accumulate in PSUM per node tile:
      S[:, 0:128] += onehot^T @ (v_em * exp(scores)_bcast),
      S[:, 128:136] += onehot^T @ exp(scores).
  - Node side: aggr = S / max(S1,eps) per head, relu, PE-transpose to
    feature-major, 2-layer output MLP, write feature-major; host untransposes.
"""
import os
import sys
from contextlib import ExitStack

import ml_dtypes
import numpy as np

sys.path.insert(0, "/opt/trn_rl_repo")

import concourse.bass as bass
import concourse.tile as tile
from concourse import mybir
from concourse.bass_utils import run_bass_kernel_spmd
from concourse.vector_clock import ScopedClock


def _patched_drain_and_barrier(self, tick_clock, wait_clock):
    # Workaround: walrus CoreV3 setupSyncWait rejects >couple sem-waits on a
    # CTRL-class (drain) instruction. Spread the tail-drain waits across
    # preceding sync-engine nops (1 wait each) and leave the drain clean.
    nc = self.nc
    nop0 = nc.sync.nop(hint="tile_drain_waits", nofuse=True)
    wait_clock.add_sem_waits(nop0.ins, ScopedClock({None: tick_clock.global_clock}))
    si = nop0.ins.sync_info
    waits = list(si.on_wait) if si is not None and si.on_wait else []
    if len(waits) > 1:
        si.on_wait = waits[:1]
        for w in waits[1:]:
            ni = nc.sync.nop(hint="tile_drain_waits", nofuse=True)
            nsi = ni.ins.sync_info
            if nsi is None:
                ni.ins.sync_info = mybir.SyncInfo(on_wait=[w], on_update=[])
            else:
                nsi.on_wait = [w]
    nc.sync.drain()
    nc.all_engine_barrier()
    popped = nc._tile_sem_poison_stack.pop()
    assert popped is self._sem_poison
    nc.clear_and_free_semaphores(list(self.sems.allocated().values()))
    nc.all_engine_barrier()


tile.TileContext._drain_and_barrier = _patched_drain_and_barrier


def _split_excess_waits(nc, max_waits=1):
    """Walrus CoreV3 setupSyncWait rejects instructions with more than one
    sem-wait. Hoist excess waits onto same-engine nops inserted just before
    the offending instruction (program order per engine is the bb order)."""
    f = nc.m.functions[0]
    offenders = {}  # name -> list of hoisted-nop Instructions
    created = set()
    for bb in f.blocks:
        for inst in bb.instructions:
            si = inst.sync_info
            if si is None or not si.on_wait or len(si.on_wait) <= max_waits:
                continue
            w = list(si.on_wait)
            nops = []
            for wt in w[:-max_waits]:
                bi = nc.engines[inst.engine].nop(nofuse=True)
                nsi = bi.ins.sync_info
                if nsi is None:
                    bi.ins.sync_info = mybir.SyncInfo(on_wait=[wt], on_update=[])
                else:
                    nsi.on_wait = [wt]
                nops.append(bi.ins)
                created.add(bi.ins.name)
            si.on_wait = w[-max_waits:]
            offenders[inst.name] = nops
    if not offenders:
        return
    for bb in f.blocks:
        insts = list(bb.instructions)
        out = []
        changed = False
        for inst in insts:
            if inst.name in created:
                changed = True
                continue
            if inst.name in offenders:
                out.extend(offenders[inst.name])
                changed = True
            out.append(inst)
        if changed:
            bb.instructions = out

# problem constants (hardcoded per contract)
N, E = 50000, 800000
SRCF, DSTF, EDGEF = 64, 64, 32
D, H, DH = 128, 8, 16
SCALE = 1.0 / np.sqrt(np.float32(DH))
NCORES = 8
P = 128
NT_TOTAL = 392
TPC = NT_TOTAL // NCORES        # 49 node tiles per core
NPC = TPC * P                   # 6272 nodes per core
SLAB_BLOCKS = 16                # edge DMA/L0 slab = 16 blocks = 2048 edges
F32 = mybir.dt.float32
I32 = mybir.dt.int32
BF16 = mybir.dt.bfloat16


# ----------------------------------------------------------------- host prep
def _prep(inputs):
    x_src = np.asarray(inputs["x_src"], np.float32)
    x_dst = np.asarray(inputs["x_dst"], np.float32)
    edge_attr = np.asarray(inputs["edge_attr"], np.float32)
    ei = np.asarray(inputs["edge_index"])
    src = ei[0].astype(np.int64)
    dst = ei[1].astype(np.int64)

    perm = np.argsort(dst, kind="stable")
    src_s, dst_s = src[perm], dst[perm]
    ea_s = edge_attr[perm]
    tile_counts = np.bincount(dst_s // P, minlength=NT_TOTAL)
    tile_starts = np.zeros(NT_TOTAL + 1, np.int64)
    np.cumsum(tile_counts, out=tile_starts[1:])

    orders = np.zeros((NCORES, TPC), np.int64)
    sorted_counts = np.zeros((NCORES, TPC), np.int64)
    for c in range(NCORES):
        tiles = np.arange(c * TPC, (c + 1) * TPC)
        o = np.argsort(-tile_counts[tiles], kind="stable")
        orders[c] = tiles[o]
        sorted_counts[c] = tile_counts[orders[c]]
    B = np.maximum(np.ceil(sorted_counts.max(axis=0) / P).astype(np.int64), 1)
    Bcum = np.zeros(TPC + 1, np.int64)
    np.cumsum(B, out=Bcum[1:])
    NBLK = int(B.sum())
    EPAD = NBLK * P

    slot = np.full((NCORES, EPAD), -1, np.int64)
    dstloc = np.full((NCORES, EPAD), -1, np.int64)
    for c in range(NCORES):
        for j in range(TPC):
            t = orders[c, j]
            s0, cnt = int(tile_starts[t]), int(tile_counts[t])
            pos = int(Bcum[j]) * P
            slot[c, pos:pos + cnt] = np.arange(s0, s0 + cnt)
            dstloc[c, pos:pos + cnt] = dst_s[s0:s0 + cnt] - t * P

    real = slot >= 0
    slot_c = np.where(real, slot, 0)
    XA = np.zeros((NCORES, 128, EPAD), ml_dtypes.bfloat16)
    XB = np.zeros((NCORES, 33, EPAD), ml_dtypes.bfloat16)
    for c in range(NCORES):
        r = real[c]
        XA[c, :64] = np.where(r, x_src[src_s[slot_c[c]]].T, 0)
        XA[c, 64:] = np.where(r, x_dst[dst_s[slot_c[c]]].T, 0)
        XB[c, :32] = np.where(r, ea_s[slot_c[c]].T, 0)
        XB[c, 32] = 1.0
    dstlocT = np.ascontiguousarray(
        dstloc.reshape(NCORES, NBLK, P).transpose(0, 2, 1)).astype(np.float32)

    kW0 = np.asarray(inputs["kW0"], np.float32)
    kb0 = np.asarray(inputs["kb0"], np.float32)
    kW1 = np.asarray(inputs["kW1"], np.float32)
    vW0 = np.asarray(inputs["vW0"], np.float32)
    vb0 = np.asarray(inputs["vb0"], np.float32)
    vW1 = np.asarray(inputs["vW1"], np.float32)
    vb1 = np.asarray(inputs["vb1"], np.float32)
    q = np.asarray(inputs["q"], np.float32)

    qmask = np.zeros((D, H), np.float32)
    for h in range(H):
        qmask[h * DH:(h + 1) * DH, h] = q[0, h * DH:(h + 1) * DH] * SCALE

    bf = ml_dtypes.bfloat16
    weights = dict(
        W0kA=np.ascontiguousarray(kW0[:128]).astype(bf),
        W0kB=np.concatenate([kW0[128:160], kb0[None, :]], 0).astype(bf),
        W0vA=np.ascontiguousarray(vW0[:128]).astype(bf),
        W0vB=np.concatenate([vW0[128:160], vb0[None, :]], 0).astype(bf),
        AQ8=((np.eye(D, dtype=np.float32) + kW1) @ qmask).astype(bf),
        MW1v=(np.eye(D, dtype=np.float32) + vW1).astype(bf),
        b1v_rep=np.tile(vb1[None, :], (P, 1)),
        oW0=np.asarray(inputs["oW0"], np.float32).astype(bf),
        ob0=np.asarray(inputs["ob0"], np.float32).reshape(P, 1),
        MoW1=(np.eye(D, dtype=np.float32)
              + np.asarray(inputs["oW1"], np.float32)).astype(bf),
        ob1=np.asarray(inputs["ob1"], np.float32).reshape(P, 1),
    )
    use_b1v = bool(np.any(weights["b1v_rep"]))
    meta = dict(B=B, Bcum=Bcum, NBLK=NBLK, EPAD=EPAD, orders=orders,
                use_b1v=use_b1v)
    staged = dict(XA=XA, XB=XB, dstlocT=dstlocT)
    return staged, weights, meta


def _unshard(out_cores, orders):
    full = np.zeros((NT_TOTAL * P, D), np.float32)
    for c in range(NCORES):
        for j in range(TPC):
            t = int(orders[c, j])
            full[t * P:(t + 1) * P] = out_cores[c][:, j * P:(j + 1) * P].T
    return np.ascontiguousarray(full[:N])


# ------------------------------------------------------------- bass program
def build_program(B, Bcum, NBLK, EPAD, use_b1v, tpc=TPC, npc=None):
    npc = npc if npc is not None else tpc * P
    nc = bass.Bass("TRN2", target_bir_lowering=False, debug=False)
    XA_d = nc.declare_dram_parameter("XA", [128, EPAD], BF16, isOutput=False)
    XB_d = nc.declare_dram_parameter("XB", [33, EPAD], BF16, isOutput=False)
    DL_d = nc.declare_dram_parameter("DL", [128, NBLK], F32, isOutput=False)
    wnames = ["W0kA", "W0kB", "W0vA", "W0vB", "AQ8", "MW1v", "b1v_rep",
              "oW0", "ob0", "MoW1", "ob1"]
    wshapes = {"W0kA": [128, 128], "W0kB": [33, 128], "W0vA": [128, 128],
               "W0vB": [33, 128], "AQ8": [128, 8], "MW1v": [128, 128],
               "b1v_rep": [128, 128], "oW0": [128, 128], "ob0": [128, 1],
               "MoW1": [128, 128], "ob1": [128, 1]}
    wdt = {"b1v_rep": F32, "ob0": F32, "ob1": F32}
    w_d = {n: nc.declare_dram_parameter(n, wshapes[n], wdt.get(n, BF16),
                                        isOutput=False)
           for n in wnames}
    OUT_d = nc.declare_dram_parameter("OUT", [128, npc], F32, isOutput=True)

    SLAB = SLAB_BLOCKS * P
    nslabs = (NBLK + SLAB_BLOCKS - 1) // SLAB_BLOCKS

    with ExitStack() as ctx:
        tc = ctx.enter_context(tile.TileContext(nc))
        cpool = ctx.enter_context(tc.tile_pool(name="consts", bufs=1))
        xpool = ctx.enter_context(tc.tile_pool(name="x", bufs=3))
        hpool = ctx.enter_context(tc.tile_pool(name="h", bufs=2))
        empool = ctx.enter_context(tc.tile_pool(name="em", bufs=3))
        npool = ctx.enter_context(tc.tile_pool(name="node", bufs=2))
        ps_l0 = ctx.enter_context(tc.tile_pool(name="psl0", bufs=1, space="PSUM"))
        ps_em = ctx.enter_context(tc.tile_pool(name="psem", bufs=2, space="PSUM"))
        ps_s = ctx.enter_context(tc.tile_pool(name="pss", bufs=1, space="PSUM"))
        ps_n = ctx.enter_context(tc.tile_pool(name="psn", bufs=1, space="PSUM"))

        # --- persistent constants ---
        w_sb = {}
        for n in wnames:
            t = cpool.tile(wshapes[n], wdt.get(n, BF16), name=f"w_{n}")
            nc.sync.dma_start(t[:], w_d[n][:])
            w_sb[n] = t
        dl_sb = cpool.tile([128, NBLK], F32, name="dl")
        nc.sync.dma_start(dl_sb[:], DL_d[:])
        iota_row_i = cpool.tile([128, 128], I32, name="iota_row_i")
        nc.gpsimd.iota(iota_row_i[:], pattern=[[1, 128]], base=0,
                       channel_multiplier=0)
        iota_row = cpool.tile([128, 128], F32, name="iota_row")
        nc.vector.tensor_copy(iota_row[:], iota_row_i[:])
        iota_p_i = cpool.tile([128, 1], I32, name="iota_p_i")
        nc.gpsimd.iota(iota_p_i[:], pattern=[[1, 1]], base=0,
                       channel_multiplier=1)
        iota_p = cpool.tile([128, 1], F32, name="iota_p")
        nc.vector.tensor_copy(iota_p[:], iota_p_i[:])
        ident_b = cpool.tile([128, 128], BF16, name="ident_b")
        nc.vector.tensor_scalar(ident_b[:], iota_row[:], iota_p[:], None,
                                op0=mybir.AluOpType.is_equal)

        headsel = np.arange(D) // DH  # feature -> head

        # --- main loop ---
        xa_t = xb_t = hk_t = hv_t = None
        slab_edges = 0
        j = 0  # current node tile
        S_ps = None
        for s in range(nslabs):
            b0 = s * SLAB_BLOCKS
            nblk_s = min(SLAB_BLOCKS, NBLK - b0)
            ne = nblk_s * P
            e0 = b0 * P
            xa_t = xpool.tile([128, SLAB], BF16, tag="xa", name=f"xa{s}")
            xb_t = xpool.tile([33, SLAB], BF16, tag="xb", name=f"xb{s}")
            nc.sync.dma_start(xa_t[:, :ne], XA_d[:, e0:e0 + ne])
            nc.sync.dma_start(xb_t[:, :ne], XB_d[:, e0:e0 + ne])
            hk_t = hpool.tile([128, SLAB], BF16, tag="hk", name=f"hk{s}")
            hv_t = hpool.tile([128, SLAB], BF16, tag="hv", name=f"hv{s}")
            # L0 in chunks of 512
            for c0 in range(0, ne, 512):
                cw = min(512, ne - c0)
                hk_ps = ps_l0.tile([128, 512], F32, tag="hkps", name=f"hkps{s}_{c0}")
                nc.tensor.matmul(hk_ps[:, :cw], w_sb["W0kA"][:],
                                 xa_t[:, c0:c0 + cw], start=True, stop=False)
                nc.tensor.matmul(hk_ps[:, :cw], w_sb["W0kB"][:],
                                 xb_t[:, c0:c0 + cw], start=False, stop=True)
                nc.scalar.activation(hk_t[:, c0:c0 + cw], hk_ps[:, :cw],
                                     mybir.ActivationFunctionType.Relu)
                hv_ps = ps_l0.tile([128, 512], F32, tag="hvps", name=f"hvps{s}_{c0}")
                nc.tensor.matmul(hv_ps[:, :cw], w_sb["W0vA"][:],
                                 xa_t[:, c0:c0 + cw], start=True, stop=False)
                nc.tensor.matmul(hv_ps[:, :cw], w_sb["W0vB"][:],
                                 xb_t[:, c0:c0 + cw], start=False, stop=True)
                nc.vector.tensor_scalar_max(hv_t[:, c0:c0 + cw], hv_ps[:, :cw],
                                            0.0)
            # em + scatter per block
            for bb in range(nblk_s):
                b = b0 + bb
                if b == Bcum[j]:
                    S_ps = ps_s.tile([128, 144], F32, tag="S", name=f"S{j}")
                first = (b == Bcum[j])
                last = (b == Bcum[j + 1] - 1)
                sl = slice(bb * P, (bb + 1) * P)

                vs_ps = ps_em.tile([128, 144], F32, tag="vs", name=f"vs{b}")
                nc.tensor.matmul(vs_ps[:, 128:136], hk_t[:, sl], w_sb["AQ8"][:],
                                 start=True, stop=True, skip_group_check=True)
                nc.tensor.matmul(vs_ps[:, 0:128], hv_t[:, sl], w_sb["MW1v"][:],
                                 start=True, stop=True, skip_group_check=True)
                ex8 = empool.tile([128, 8], F32, tag="ex8", name=f"ex8_{b}")
                nc.scalar.activation(ex8[:], vs_ps[:, 128:136],
                                     mybir.ActivationFunctionType.Exp)
                exvs = empool.tile([128, 136], BF16, tag="exvs", name=f"exvs{b}")
                nc.vector.tensor_copy(exvs[:, 128:136], ex8[:])
                if use_b1v:
                    vtmp = empool.tile([128, 128], F32, tag="vtmp", name=f"vt{b}")
                    nc.vector.tensor_add(vtmp[:], vs_ps[:, 0:128],
                                         w_sb["b1v_rep"][:])
                    vsrc = vtmp
                else:
                    vsrc = vs_ps
                vap = (vsrc[:, 0:128] if not use_b1v else vsrc[:])
                nc.vector.tensor_tensor(
                    exvs[:, 0:128].rearrange("p (h r) -> p h r", r=DH),
                    vap.rearrange("p (h r) -> p h r", r=DH),
                    ex8[:].unsqueeze(2).broadcast_to([128, 8, DH]),
                    op=mybir.AluOpType.mult)
                oh = empool.tile([128, 128], BF16, tag="oh", name=f"oh{b}")
                nc.vector.tensor_scalar(oh[:], iota_row[:], dl_sb[:, b:b + 1],
                                        None, op0=mybir.AluOpType.is_equal)
                nc.tensor.matmul(S_ps[:, 0:136], oh[:], exvs[:],
                                 start=first, stop=last, skip_group_check=True)

                if last:
                    # node-tile epilogue
                    s1 = npool.tile([128, 8], F32, tag="s1", name=f"s1_{j}")
                    nc.vector.tensor_scalar_max(s1[:], S_ps[:, 128:136], 1e-30)
                    r1 = npool.tile([128, 8], F32, tag="r1", name=f"r1_{j}")
                    nc.vector.reciprocal(r1[:], s1[:])
                    g = npool.tile([128, 128], BF16, tag="g", name=f"g{j}")
                    nc.vector.tensor_tensor(
                        g[:].rearrange("p (h r) -> p h r", r=DH),
                        S_ps[:, 0:128].rearrange("p (h r) -> p h r", r=DH),
                        r1[:].unsqueeze(2).broadcast_to([128, 8, DH]),
                        op=mybir.AluOpType.mult)
                    nc.vector.tensor_scalar_max(g[:], g[:], 0.0)
                    tp_ps = ps_n.tile([128, 128], BF16, tag="npsb",
                                      name=f"tp{j}")
                    nc.tensor.transpose(tp_ps[:], g[:], ident_b[:])
                    gfm = npool.tile([128, 128], BF16, tag="gfm", name=f"gfm{j}")
                    nc.scalar.copy(gfm[:], tp_ps[:])
                    h0_ps = ps_n.tile([128, 128], F32, tag="nps", name=f"h0p{j}")
                    nc.tensor.matmul(h0_ps[:], w_sb["oW0"][:], gfm[:],
                                     start=True, stop=True)
                    h0 = npool.tile([128, 128], BF16, tag="h0", name=f"h0{j}")
                    nc.scalar.activation(h0[:], h0_ps[:],
                                         mybir.ActivationFunctionType.Relu,
                                         bias=w_sb["ob0"][:])
                    o2_ps = ps_n.tile([128, 128], F32, tag="nps", name=f"o2p{j}")
                    nc.tensor.matmul(o2_ps[:], w_sb["MoW1"][:], h0[:],
                                     start=True, stop=True)
                    ot = npool.tile([128, 128], F32, tag="ot", name=f"ot{j}")
                    nc.scalar.activation(ot[:], o2_ps[:],
                                         mybir.ActivationFunctionType.Relu,
                                         bias=w_sb["ob1"][:])
                    nc.sync.dma_start(OUT_d[:, j * P:(j + 1) * P], ot[:])
                    j += 1
    _split_excess_waits(nc)
    return nc


# ------------------------------------------------------------------ kernel
def kernel(**inputs):
    staged, weights, meta = _prep(inputs)
    nc = build_program(meta["B"], meta["Bcum"], meta["NBLK"], meta["EPAD"],
                       meta["use_b1v"])
    in_maps = []
    for c in range(NCORES):
        m = {"XA": staged["XA"][c], "XB": staged["XB"][c],
             "DL": staged["dstlocT"][c]}
        m.update(weights)
        in_maps.append(m)
    res = run_bass_kernel_spmd(nc, in_maps, list(range(NCORES)))
    out_cores = [res.results[c]["OUT"] for c in range(NCORES)]
    return _unshard(out_cores, meta["orders"])


# revision 19
# speedup vs baseline: 2.8905x; 1.2030x over previous
"""Trainium2 Bass kernel for nn_NeighborhoodAttention (GNN message passing).

Strategy (no collectives needed):
  - Host: sort edges by dst, pad nodes 50000->50176 = 392 tiles of 128; core c
    owns 49 contiguous node tiles and their (contiguous) edges. Per node tile,
    the edge list is padded to a multiple of 128 ("blocks"); within each core,
    tiles are processed in descending-edge-count order so the per-position
    block count B_j is shared across all 8 cores (single SPMD program).
  - Gathered endpoint features are staged feature-major ([feat, edge]); the
    per-edge MLPs run feature-major on TensorE with stationary weights.
    Residual layers and the q-dot are folded on the host:
      scores = ((I+kW1) @ qmask)^T h_k        (per-head additive consts cancel
      v      = (I+vW1)^T h_v + vb1             exactly in the segment softmax,
                                               incl. the max subtraction)
  - Edge-major tensors (scores_em, v_em) come from "em-mode" matmuls using the
    h-slab slice as the stationary operand - no transposes anywhere.
  - Segment softmax/scatter-add: per 128-edge block build onehot[e, n] on DVE
    (iota + is_equal vs dst_local), then accumulate in PSUM per node tile:
      S[:, 0:128] += onehot^T @ (v_em * exp(scores)_bcast),
      S[:, 128:136] += onehot^T @ exp(scores).
  - Node side: aggr = S / max(S1,eps) per head, relu, PE-transpose to
    feature-major, 2-layer output MLP, write feature-major; host untransposes.
"""
import os
import sys
from contextlib import ExitStack

import ml_dtypes
import numpy as np

sys.path.insert(0, "/opt/trn_rl_repo")

import concourse.bass as bass
import concourse.tile as tile
from concourse import mybir
from concourse.bass_utils import run_bass_kernel_spmd
from concourse.vector_clock import ScopedClock


def _patched_drain_and_barrier(self, tick_clock, wait_clock):
    # Workaround: walrus CoreV3 setupSyncWait rejects >couple sem-waits on a
    # CTRL-class (drain) instruction. Spread the tail-drain waits across
    # preceding sync-engine nops (1 wait each) and leave the drain clean.
    nc = self.nc
    nop0 = nc.sync.nop(hint="tile_drain_waits", nofuse=True)
    wait_clock.add_sem_waits(nop0.ins, ScopedClock({None: tick_clock.global_clock}))
    si = nop0.ins.sync_info
    waits = list(si.on_wait) if si is not None and si.on_wait else []
    if len(waits) > 1:
        si.on_wait = waits[:1]
        for w in waits[1:]:
            ni = nc.sync.nop(hint="tile_drain_waits", nofuse=True)
            nsi = ni.ins.sync_info
            if nsi is None:
                ni.ins.sync_info = mybir.SyncInfo(on_wait=[w], on_update=[])
            else:
                nsi.on_wait = [w]
    nc.sync.drain()
    nc.all_engine_barrier()
    popped = nc._tile_sem_poison_stack.pop()
    assert popped is self._sem_poison
    nc.clear_and_free_semaphores(list(self.sems.allocated().values()))
    nc.all_engine_barrier()


tile.TileContext._drain_and_barrier = _patched_drain_and_barrier


def _split_excess_waits(nc, max_waits=1):
    """Walrus CoreV3 setupSyncWait rejects instructions with more than one
    sem-wait. Hoist excess waits onto same-engine nops inserted just before
    the offending instruction (program order per engine is the bb order)."""
    f = nc.m.functions[0]
    offenders = {}  # name -> list of hoisted-nop Instructions
    created = set()
    for bb in f.blocks:
        for inst in bb.instructions:
            si = inst.sync_info
            if si is None or not si.on_wait or len(si.on_wait) <= max_waits:
                continue
            w = list(si.on_wait)
            nops = []
            for wt in w[:-max_waits]:
                bi = nc.engines[inst.engine].nop(nofuse=True)
                nsi = bi.ins.sync_info
                if nsi is None:
                    bi.ins.sync_info = mybir.SyncInfo(on_wait=[wt], on_update=[])
                else:
                    nsi.on_wait = [wt]
                nops.append(bi.ins)
                created.add(bi.ins.name)
            si.on_wait = w[-max_waits:]
            offenders[inst.name] = nops
    if not offenders:
        return
    for bb in f.blocks:
        insts = list(bb.instructions)
        out = []
        changed = False
        for inst in insts:
            if inst.name in created:
                changed = True
                continue
            if inst.name in offenders:
                out.extend(offenders[inst.name])
                changed = True
            out.append(inst)
        if changed:
            bb.instructions = out

# problem constants (hardcoded per contract)
N, E = 50000, 800000
SRCF, DSTF, EDGEF = 64, 64, 32
D, H, DH = 128, 8, 16
SCALE = 1.0 / np.sqrt(np.float32(DH))
NCORES = 8
P = 128
NT_TOTAL = 392
TPC = NT_TOTAL // NCORES        # 49 node tiles per core
NPC = TPC * P                   # 6272 nodes per core
SLAB_BLOCKS = 16                # edge DMA/L0 slab = 16 blocks = 2048 edges
F32 = mybir.dt.float32
I32 = mybir.dt.int32
BF16 = mybir.dt.bfloat16


# ----------------------------------------------------------------- host prep
def _prep(inputs):
    x_src = np.asarray(inputs["x_src"], np.float32)
    x_dst = np.asarray(inputs["x_dst"], np.float32)
    edge_attr = np.asarray(inputs["edge_attr"], np.float32)
    ei = np.asarray(inputs["edge_index"])
    src = ei[0].astype(np.int64)
    dst = ei[1].astype(np.int64)

    perm = np.argsort(dst, kind="stable")
    src_s, dst_s = src[perm], dst[perm]
    ea_s = edge_attr[perm]
    tile_counts = np.bincount(dst_s // P, minlength=NT_TOTAL)
    tile_starts = np.zeros(NT_TOTAL + 1, np.int64)
    np.cumsum(tile_counts, out=tile_starts[1:])

    orders = np.zeros((NCORES, TPC), np.int64)
    sorted_counts = np.zeros((NCORES, TPC), np.int64)
    for c in range(NCORES):
        tiles = np.arange(c * TPC, (c + 1) * TPC)
        o = np.argsort(-tile_counts[tiles], kind="stable")
        orders[c] = tiles[o]
        sorted_counts[c] = tile_counts[orders[c]]
    B = np.maximum(np.ceil(sorted_counts.max(axis=0) / P).astype(np.int64), 1)
    Bcum = np.zeros(TPC + 1, np.int64)
    np.cumsum(B, out=Bcum[1:])
    NBLK = int(B.sum())
    EPAD = NBLK * P

    slot = np.full((NCORES, EPAD), -1, np.int64)
    dstloc = np.full((NCORES, EPAD), -1, np.int64)
    for c in range(NCORES):
        for j in range(TPC):
            t = orders[c, j]
            s0, cnt = int(tile_starts[t]), int(tile_counts[t])
            pos = int(Bcum[j]) * P
            slot[c, pos:pos + cnt] = np.arange(s0, s0 + cnt)
            dstloc[c, pos:pos + cnt] = dst_s[s0:s0 + cnt] - t * P

    real = slot >= 0
    slot_c = np.where(real, slot, 0)
    XA = np.zeros((NCORES, 128, EPAD), ml_dtypes.bfloat16)
    XB = np.zeros((NCORES, 33, EPAD), ml_dtypes.bfloat16)
    for c in range(NCORES):
        r = real[c]
        XA[c, :64] = np.where(r, x_src[src_s[slot_c[c]]].T, 0)
        XA[c, 64:] = np.where(r, x_dst[dst_s[slot_c[c]]].T, 0)
        XB[c, :32] = np.where(r, ea_s[slot_c[c]].T, 0)
        XB[c, 32] = 1.0
    dstlocT = np.ascontiguousarray(
        dstloc.reshape(NCORES, NBLK, P).transpose(0, 2, 1)).astype(np.float32)

    kW0 = np.asarray(inputs["kW0"], np.float32)
    kb0 = np.asarray(inputs["kb0"], np.float32)
    kW1 = np.asarray(inputs["kW1"], np.float32)
    vW0 = np.asarray(inputs["vW0"], np.float32)
    vb0 = np.asarray(inputs["vb0"], np.float32)
    vW1 = np.asarray(inputs["vW1"], np.float32)
    vb1 = np.asarray(inputs["vb1"], np.float32)
    q = np.asarray(inputs["q"], np.float32)

    qmask = np.zeros((D, H), np.float32)
    for h in range(H):
        qmask[h * DH:(h + 1) * DH, h] = q[0, h * DH:(h + 1) * DH] * SCALE

    bf = ml_dtypes.bfloat16
    weights = dict(
        W0kA=np.ascontiguousarray(kW0[:128]).astype(bf),
        W0kB=np.concatenate([kW0[128:160], kb0[None, :]], 0).astype(bf),
        W0vA=np.ascontiguousarray(vW0[:128]).astype(bf),
        W0vB=np.concatenate([vW0[128:160], vb0[None, :]], 0).astype(bf),
        AQ8=((np.eye(D, dtype=np.float32) + kW1) @ qmask).astype(bf),
        MW1v=(np.eye(D, dtype=np.float32) + vW1).astype(bf),
        b1v_rep=np.tile(vb1[None, :], (P, 1)),
        oW0=np.asarray(inputs["oW0"], np.float32).astype(bf),
        ob0=np.asarray(inputs["ob0"], np.float32).reshape(P, 1),
        MoW1=(np.eye(D, dtype=np.float32)
              + np.asarray(inputs["oW1"], np.float32)).astype(bf),
        ob1=np.asarray(inputs["ob1"], np.float32).reshape(P, 1),
    )
    use_b1v = bool(np.any(weights["b1v_rep"]))
    meta = dict(B=B, Bcum=Bcum, NBLK=NBLK, EPAD=EPAD, orders=orders,
                use_b1v=use_b1v)
    staged = dict(XA=XA, XB=XB, dstlocT=dstlocT)
    return staged, weights, meta


def _unshard(out_cores, orders):
    full = np.zeros((NT_TOTAL * P, D), np.float32)
    for c in range(NCORES):
        for j in range(TPC):
            t = int(orders[c, j])
            full[t * P:(t + 1) * P] = out_cores[c][:, j * P:(j + 1) * P].T
    return np.ascontiguousarray(full[:N])


# ------------------------------------------------------------- bass program
def build_program(B, Bcum, NBLK, EPAD, use_b1v, tpc=TPC, npc=None):
    npc = npc if npc is not None else tpc * P
    nc = bass.Bass("TRN2", target_bir_lowering=False, debug=False)
    XA_d = nc.declare_dram_parameter("XA", [128, EPAD], BF16, isOutput=False)
    XB_d = nc.declare_dram_parameter("XB", [33, EPAD], BF16, isOutput=False)
    DL_d = nc.declare_dram_parameter("DL", [128, NBLK], F32, isOutput=False)
    wnames = ["W0kA", "W0kB", "W0vA", "W0vB", "AQ8", "MW1v", "b1v_rep",
              "oW0", "ob0", "MoW1", "ob1"]
    wshapes = {"W0kA": [128, 128], "W0kB": [33, 128], "W0vA": [128, 128],
               "W0vB": [33, 128], "AQ8": [128, 8], "MW1v": [128, 128],
               "b1v_rep": [128, 128], "oW0": [128, 128], "ob0": [128, 1],
               "MoW1": [128, 128], "ob1": [128, 1]}
    wdt = {"b1v_rep": F32, "ob0": F32, "ob1": F32}
    w_d = {n: nc.declare_dram_parameter(n, wshapes[n], wdt.get(n, BF16),
                                        isOutput=False)
           for n in wnames}
    OUT_d = nc.declare_dram_parameter("OUT", [128, npc], F32, isOutput=True)

    SLAB = SLAB_BLOCKS * P
    nslabs = (NBLK + SLAB_BLOCKS - 1) // SLAB_BLOCKS

    with ExitStack() as ctx:
        tc = ctx.enter_context(tile.TileContext(nc))
        cpool = ctx.enter_context(tc.tile_pool(name="consts", bufs=1))
        xpool = ctx.enter_context(tc.tile_pool(name="x", bufs=3))
        hpool = ctx.enter_context(tc.tile_pool(name="h", bufs=2))
        ohpool = ctx.enter_context(tc.tile_pool(name="ohp", bufs=2))
        expool = ctx.enter_context(tc.tile_pool(name="exp", bufs=2))
        empool = ctx.enter_context(tc.tile_pool(name="em", bufs=6))
        npool = ctx.enter_context(tc.tile_pool(name="node", bufs=2))
        ps_l0 = ctx.enter_context(tc.tile_pool(name="psl0", bufs=1, space="PSUM"))
        ps_sc = ctx.enter_context(tc.tile_pool(name="pssc", bufs=1, space="PSUM"))
        ps_v = ctx.enter_context(tc.tile_pool(name="psv", bufs=2, space="PSUM"))
        ps_s = ctx.enter_context(tc.tile_pool(name="pss", bufs=1, space="PSUM"))
        ps_n = ctx.enter_context(tc.tile_pool(name="psn", bufs=1, space="PSUM"))

        # --- persistent constants ---
        w_sb = {}
        for n in wnames:
            t = cpool.tile(wshapes[n], wdt.get(n, BF16), name=f"w_{n}")
            nc.sync.dma_start(t[:], w_d[n][:])
            w_sb[n] = t
        dl_sb = cpool.tile([128, NBLK], F32, name="dl")
        nc.sync.dma_start(dl_sb[:], DL_d[:])
        iota_row_i = cpool.tile([128, 128], I32, name="iota_row_i")
        nc.gpsimd.iota(iota_row_i[:], pattern=[[1, 128]], base=0,
                       channel_multiplier=0)
        iota_row = cpool.tile([128, 128], F32, name="iota_row")
        nc.vector.tensor_copy(iota_row[:], iota_row_i[:])
        iota_p_i = cpool.tile([128, 1], I32, name="iota_p_i")
        nc.gpsimd.iota(iota_p_i[:], pattern=[[1, 1]], base=0,
                       channel_multiplier=1)
        iota_p = cpool.tile([128, 1], F32, name="iota_p")
        nc.vector.tensor_copy(iota_p[:], iota_p_i[:])
        ident_b = cpool.tile([128, 128], BF16, name="ident_b")
        nc.vector.tensor_scalar(ident_b[:], iota_row[:], iota_p[:], None,
                                op0=mybir.AluOpType.is_equal)

        headsel = np.arange(D) // DH  # feature -> head

        # --- main loop ---
        xa_t = xb_t = hk_t = hv_t = None
        slab_edges = 0
        j = 0  # current node tile
        S_ps = None
        for s in range(nslabs):
            b0 = s * SLAB_BLOCKS
            nblk_s = min(SLAB_BLOCKS, NBLK - b0)
            ne = nblk_s * P
            e0 = b0 * P
            xa_t = xpool.tile([128, SLAB], BF16, tag="xa", name=f"xa{s}")
            xb_t = xpool.tile([33, SLAB], BF16, tag="xb", name=f"xb{s}")
            nc.sync.dma_start(xa_t[:, :ne], XA_d[:, e0:e0 + ne])
            nc.sync.dma_start(xb_t[:, :ne], XB_d[:, e0:e0 + ne])
            hk_t = hpool.tile([128, SLAB], BF16, tag="hk", name=f"hk{s}")
            hv_t = hpool.tile([128, SLAB], BF16, tag="hv", name=f"hv{s}")
            # L0 in chunks of 512
            for c0 in range(0, ne, 512):
                cw = min(512, ne - c0)
                hk_ps = ps_l0.tile([128, 512], F32, tag="hkps", name=f"hkps{s}_{c0}")
                nc.tensor.matmul(hk_ps[:, :cw], w_sb["W0kA"][:],
                                 xa_t[:, c0:c0 + cw], start=True, stop=False)
                nc.tensor.matmul(hk_ps[:, :cw], w_sb["W0kB"][:],
                                 xb_t[:, c0:c0 + cw], start=False, stop=True)
                nc.scalar.activation(hk_t[:, c0:c0 + cw], hk_ps[:, :cw],
                                     mybir.ActivationFunctionType.Relu)
                hv_ps = ps_l0.tile([128, 512], F32, tag="hvps", name=f"hvps{s}_{c0}")
                nc.tensor.matmul(hv_ps[:, :cw], w_sb["W0vA"][:],
                                 xa_t[:, c0:c0 + cw], start=True, stop=False)
                nc.tensor.matmul(hv_ps[:, :cw], w_sb["W0vB"][:],
                                 xb_t[:, c0:c0 + cw], start=False, stop=True)
                nc.vector.tensor_scalar_max(hv_t[:, c0:c0 + cw], hv_ps[:, :cw],
                                            0.0)
            # one-hot for the whole slab (bf16): oh[p, 128*bb+n] = (dl==n)
            oh_t = ohpool.tile([128, SLAB_BLOCKS, P], BF16, tag="oh",
                               name=f"oh{s}")
            nc.vector.tensor_tensor(
                oh_t[:, :nblk_s, :],
                iota_row[:].unsqueeze(1).broadcast_to([128, nblk_s, P]),
                dl_sb[:, b0:b0 + nblk_s].unsqueeze(2).broadcast_to(
                    [128, nblk_s, P]),
                op=mybir.AluOpType.is_equal)
            # scores for the whole slab -> single exp
            sc_ps = ps_sc.tile([128, SLAB_BLOCKS, 8], F32, tag="sc",
                               name=f"sc{s}")
            for bb in range(nblk_s):
                nc.tensor.matmul(sc_ps[:, bb, :],
                                 hk_t[:, bb * P:(bb + 1) * P], w_sb["AQ8"][:],
                                 start=True, stop=True, skip_group_check=True)
            ex_t = expool.tile([128, SLAB_BLOCKS, 8], F32, tag="ex",
                               name=f"ex{s}")
            nc.scalar.activation(ex_t[:, :nblk_s, :], sc_ps[:, :nblk_s, :],
                                 mybir.ActivationFunctionType.Exp)
            # v matmuls per quad + batched exv mult / ex8 cast
            exvs_at = {}
            for q0 in range(0, nblk_s, 4):
                qn = min(4, nblk_s - q0)
                v_ps = ps_v.tile([128, 4, 128], F32, tag="vps",
                                 name=f"vp{s}_{q0}")
                for i in range(qn):
                    bb = q0 + i
                    nc.tensor.matmul(v_ps[:, i, :],
                                     hv_t[:, bb * P:(bb + 1) * P],
                                     w_sb["MW1v"][:], start=True, stop=True,
                                     skip_group_check=True)
                if use_b1v:
                    nc.vector.tensor_tensor(
                        v_ps[:, :qn, :],
                        v_ps[:, :qn, :],
                        w_sb["b1v_rep"][:].unsqueeze(1).broadcast_to(
                            [128, qn, 128]),
                        op=mybir.AluOpType.add)
                exvs = empool.tile([128, 4, 136], BF16, tag="exvs",
                                   name=f"exvs{s}_{q0}")
                nc.vector.tensor_copy(
                    exvs[:, :qn, 128:136],
                    ex_t[:, q0:q0 + qn, :])
                nc.vector.tensor_tensor(
                    exvs[:, :qn, 0:128].rearrange("p q (h r) -> p q h r", r=DH),
                    v_ps[:, :qn, :].rearrange("p q (h r) -> p q h r", r=DH),
                    ex_t[:, q0:q0 + qn, :].unsqueeze(3).broadcast_to(
                        [128, qn, 8, DH]),
                    op=mybir.AluOpType.mult)
                for i in range(qn):
                    exvs_at[q0 + i] = (exvs, i)
            # scatter + node epilogues
            for bb in range(nblk_s):
                b = b0 + bb
                if b == Bcum[j]:
                    S_ps = ps_s.tile([128, 144], F32, tag="S", name=f"S{j}")
                first = (b == Bcum[j])
                last = (b == Bcum[j + 1] - 1)
                exvs, i = exvs_at[bb]
                nc.tensor.matmul(S_ps[:, 0:136], oh_t[:, bb, :],
                                 exvs[:, i, :], start=first, stop=last,
                                 skip_group_check=True)

                if last:
                    # node-tile epilogue
                    s1 = npool.tile([128, 8], F32, tag="s1", name=f"s1_{j}")
                    nc.vector.tensor_scalar_max(s1[:], S_ps[:, 128:136], 1e-30)
                    r1 = npool.tile([128, 8], F32, tag="r1", name=f"r1_{j}")
                    nc.vector.reciprocal(r1[:], s1[:])
                    g = npool.tile([128, 128], BF16, tag="g", name=f"g{j}")
                    nc.vector.tensor_tensor(
                        g[:].rearrange("p (h r) -> p h r", r=DH),
                        S_ps[:, 0:128].rearrange("p (h r) -> p h r", r=DH),
                        r1[:].unsqueeze(2).broadcast_to([128, 8, DH]),
                        op=mybir.AluOpType.mult)
                    nc.vector.tensor_scalar_max(g[:], g[:], 0.0)
                    tp_ps = ps_n.tile([128, 128], BF16, tag="npsb",
                                      name=f"tp{j}")
                    nc.tensor.transpose(tp_ps[:], g[:], ident_b[:])
                    gfm = npool.tile([128, 128], BF16, tag="gfm", name=f"gfm{j}")
                    nc.scalar.copy(gfm[:], tp_ps[:])
                    h0_ps = ps_n.tile([128, 128], F32, tag="nps", name=f"h0p{j}")
                    nc.tensor.matmul(h0_ps[:], w_sb["oW0"][:], gfm[:],
                                     start=True, stop=True)
                    h0 = npool.tile([128, 128], BF16, tag="h0", name=f"h0{j}")
                    nc.scalar.activation(h0[:], h0_ps[:],
                                         mybir.ActivationFunctionType.Relu,
                                         bias=w_sb["ob0"][:])
                    o2_ps = ps_n.tile([128, 128], F32, tag="nps", name=f"o2p{j}")
                    nc.tensor.matmul(o2_ps[:], w_sb["MoW1"][:], h0[:],
                                     start=True, stop=True)
                    ot = npool.tile([128, 128], F32, tag="ot", name=f"ot{j}")
                    nc.scalar.activation(ot[:], o2_ps[:],
                                         mybir.ActivationFunctionType.Relu,
                                         bias=w_sb["ob1"][:])
                    nc.sync.dma_start(OUT_d[:, j * P:(j + 1) * P], ot[:])
                    j += 1
    _split_excess_waits(nc)
    return nc


# ------------------------------------------------------------------ kernel
def kernel(**inputs):
    staged, weights, meta = _prep(inputs)
    nc = build_program(meta["B"], meta["Bcum"], meta["NBLK"], meta["EPAD"],
                       meta["use_b1v"])
    in_maps = []
    for c in range(NCORES):
        m = {"XA": staged["XA"][c], "XB": staged["XB"][c],
             "DL": staged["dstlocT"][c]}
        m.update(weights)
        in_maps.append(m)
    res = run_bass_kernel_spmd(nc, in_maps, list(range(NCORES)))
    out_cores = [res.results[c]["OUT"] for c in range(NCORES)]
    return _unshard(out_cores, meta["orders"])


# revision 21
# speedup vs baseline: 3.5974x; 1.2446x over previous
"""Trainium2 Bass kernel for nn_NeighborhoodAttention (GNN message passing).

Strategy (no collectives needed):
  - Host: sort edges by dst, pad nodes 50000->50176 = 392 tiles of 128; core c
    owns 49 contiguous node tiles and their (contiguous) edges. Per node tile,
    the edge list is padded to a multiple of 128 ("blocks"); within each core,
    tiles are processed in descending-edge-count order so the per-position
    block count B_j is shared across all 8 cores (single SPMD program).
  - Gathered endpoint features are staged feature-major ([feat, edge]); the
    per-edge MLPs run feature-major on TensorE with stationary weights.
    Residual layers and the q-dot are folded on the host:
      scores = ((I+kW1) @ qmask)^T h_k        (per-head additive consts cancel
      v      = (I+vW1)^T h_v + vb1             exactly in the segment softmax,
                                               incl. the max subtraction)
  - Edge-major tensors (scores_em, v_em) come from "em-mode" matmuls using the
    h-slab slice as the stationary operand - no transposes anywhere.
  - Segment softmax/scatter-add: per 128-edge block build onehot[e, n] on DVE
    (iota + is_equal vs dst_local), then accumulate in PSUM per node tile:
      S[:, 0:128] += onehot^T @ (v_em * exp(scores)_bcast),
      S[:, 128:136] += onehot^T @ exp(scores).
  - Node side: aggr = S / max(S1,eps) per head, relu, PE-transpose to
    feature-major, 2-layer output MLP, write feature-major; host untransposes.
"""
import os
import sys
from contextlib import ExitStack

import ml_dtypes
import numpy as np

sys.path.insert(0, "/opt/trn_rl_repo")

import concourse.bass as bass
import concourse.tile as tile
from concourse import mybir
from concourse.bass_utils import run_bass_kernel_spmd
from concourse.vector_clock import ScopedClock


def _patched_drain_and_barrier(self, tick_clock, wait_clock):
    # Workaround: walrus CoreV3 setupSyncWait rejects >couple sem-waits on a
    # CTRL-class (drain) instruction. Spread the tail-drain waits across
    # preceding sync-engine nops (1 wait each) and leave the drain clean.
    nc = self.nc
    nop0 = nc.sync.nop(hint="tile_drain_waits", nofuse=True)
    wait_clock.add_sem_waits(nop0.ins, ScopedClock({None: tick_clock.global_clock}))
    si = nop0.ins.sync_info
    waits = list(si.on_wait) if si is not None and si.on_wait else []
    if len(waits) > 1:
        si.on_wait = waits[:1]
        for w in waits[1:]:
            ni = nc.sync.nop(hint="tile_drain_waits", nofuse=True)
            nsi = ni.ins.sync_info
            if nsi is None:
                ni.ins.sync_info = mybir.SyncInfo(on_wait=[w], on_update=[])
            else:
                nsi.on_wait = [w]
    nc.sync.drain()
    nc.all_engine_barrier()
    popped = nc._tile_sem_poison_stack.pop()
    assert popped is self._sem_poison
    nc.clear_and_free_semaphores(list(self.sems.allocated().values()))
    nc.all_engine_barrier()


tile.TileContext._drain_and_barrier = _patched_drain_and_barrier


def _split_excess_waits(nc, max_waits=1):
    """Walrus CoreV3 setupSyncWait rejects instructions with more than one
    sem-wait. Hoist excess waits onto same-engine nops inserted just before
    the offending instruction (program order per engine is the bb order)."""
    f = nc.m.functions[0]
    offenders = {}  # name -> list of hoisted-nop Instructions
    created = set()
    for bb in f.blocks:
        for inst in bb.instructions:
            si = inst.sync_info
            if si is None or not si.on_wait or len(si.on_wait) <= max_waits:
                continue
            w = list(si.on_wait)
            nops = []
            for wt in w[:-max_waits]:
                bi = nc.engines[inst.engine].nop(nofuse=True)
                nsi = bi.ins.sync_info
                if nsi is None:
                    bi.ins.sync_info = mybir.SyncInfo(on_wait=[wt], on_update=[])
                else:
                    nsi.on_wait = [wt]
                nops.append(bi.ins)
                created.add(bi.ins.name)
            si.on_wait = w[-max_waits:]
            offenders[inst.name] = nops
    if not offenders:
        return
    for bb in f.blocks:
        insts = list(bb.instructions)
        out = []
        changed = False
        for inst in insts:
            if inst.name in created:
                changed = True
                continue
            if inst.name in offenders:
                out.extend(offenders[inst.name])
                changed = True
            out.append(inst)
        if changed:
            bb.instructions = out

# problem constants (hardcoded per contract)
N, E = 50000, 800000
SRCF, DSTF, EDGEF = 64, 64, 32
D, H, DH = 128, 8, 16
SCALE = 1.0 / np.sqrt(np.float32(DH))
NCORES = 8
P = 128
NT_TOTAL = 392
TPC = NT_TOTAL // NCORES        # 49 node tiles per core
NPC = TPC * P                   # 6272 nodes per core
SLAB_BLOCKS = 16                # edge DMA/L0 slab = 16 blocks = 2048 edges
F32 = mybir.dt.float32
I32 = mybir.dt.int32
BF16 = mybir.dt.bfloat16


# ----------------------------------------------------------------- host prep
def _prep(inputs):
    x_src = np.asarray(inputs["x_src"], np.float32)
    x_dst = np.asarray(inputs["x_dst"], np.float32)
    edge_attr = np.asarray(inputs["edge_attr"], np.float32)
    ei = np.asarray(inputs["edge_index"])
    src = ei[0].astype(np.int64)
    dst = ei[1].astype(np.int64)

    perm = np.argsort(dst, kind="stable")
    src_s, dst_s = src[perm], dst[perm]
    ea_s = edge_attr[perm]
    tile_counts = np.bincount(dst_s // P, minlength=NT_TOTAL)
    tile_starts = np.zeros(NT_TOTAL + 1, np.int64)
    np.cumsum(tile_counts, out=tile_starts[1:])

    orders = np.zeros((NCORES, TPC), np.int64)
    sorted_counts = np.zeros((NCORES, TPC), np.int64)
    for c in range(NCORES):
        tiles = np.arange(c * TPC, (c + 1) * TPC)
        o = np.argsort(-tile_counts[tiles], kind="stable")
        orders[c] = tiles[o]
        sorted_counts[c] = tile_counts[orders[c]]
    B = np.maximum(np.ceil(sorted_counts.max(axis=0) / P).astype(np.int64), 1)
    Bcum = np.zeros(TPC + 1, np.int64)
    np.cumsum(B, out=Bcum[1:])
    NBLK = int(B.sum())
    EPAD = NBLK * P

    slot = np.full((NCORES, EPAD), -1, np.int64)
    dstloc = np.full((NCORES, EPAD), -1, np.int64)
    for c in range(NCORES):
        for j in range(TPC):
            t = orders[c, j]
            s0, cnt = int(tile_starts[t]), int(tile_counts[t])
            pos = int(Bcum[j]) * P
            slot[c, pos:pos + cnt] = np.arange(s0, s0 + cnt)
            dstloc[c, pos:pos + cnt] = dst_s[s0:s0 + cnt] - t * P

    real = slot >= 0
    slot_c = np.where(real, slot, 0)
    XA = np.zeros((NCORES, 128, EPAD), ml_dtypes.bfloat16)
    XB = np.zeros((NCORES, 33, EPAD), ml_dtypes.bfloat16)
    for c in range(NCORES):
        r = real[c]
        XA[c, :64] = np.where(r, x_src[src_s[slot_c[c]]].T, 0)
        XA[c, 64:] = np.where(r, x_dst[dst_s[slot_c[c]]].T, 0)
        XB[c, :32] = np.where(r, ea_s[slot_c[c]].T, 0)
        XB[c, 32] = 1.0
    dstlocT = np.ascontiguousarray(
        dstloc.reshape(NCORES, NBLK, P).transpose(0, 2, 1)).astype(np.float32)

    kW0 = np.asarray(inputs["kW0"], np.float32)
    kb0 = np.asarray(inputs["kb0"], np.float32)
    kW1 = np.asarray(inputs["kW1"], np.float32)
    vW0 = np.asarray(inputs["vW0"], np.float32)
    vb0 = np.asarray(inputs["vb0"], np.float32)
    vW1 = np.asarray(inputs["vW1"], np.float32)
    vb1 = np.asarray(inputs["vb1"], np.float32)
    q = np.asarray(inputs["q"], np.float32)

    qmask = np.zeros((D, H), np.float32)
    for h in range(H):
        qmask[h * DH:(h + 1) * DH, h] = q[0, h * DH:(h + 1) * DH] * SCALE

    bf = ml_dtypes.bfloat16
    weights = dict(
        W0kA=np.ascontiguousarray(kW0[:128]).astype(bf),
        W0kB=np.concatenate([kW0[128:160], kb0[None, :]], 0).astype(bf),
        W0vA=np.ascontiguousarray(vW0[:128]).astype(bf),
        W0vB=np.concatenate([vW0[128:160], vb0[None, :]], 0).astype(bf),
        AQ8=((np.eye(D, dtype=np.float32) + kW1) @ qmask).astype(bf),
        MW1v=(np.eye(D, dtype=np.float32) + vW1).astype(bf),
        b1v_rep=np.tile(vb1[None, :], (P, 1)),
        oW0=np.asarray(inputs["oW0"], np.float32).astype(bf),
        ob0=np.asarray(inputs["ob0"], np.float32).reshape(P, 1),
        MoW1=(np.eye(D, dtype=np.float32)
              + np.asarray(inputs["oW1"], np.float32)).astype(bf),
        ob1=np.asarray(inputs["ob1"], np.float32).reshape(P, 1),
    )
    use_b1v = bool(np.any(weights["b1v_rep"]))
    meta = dict(B=B, Bcum=Bcum, NBLK=NBLK, EPAD=EPAD, orders=orders,
                use_b1v=use_b1v)
    staged = dict(XA=XA, XB=XB, dstlocT=dstlocT)
    return staged, weights, meta


def _unshard(out_cores, orders):
    full = np.zeros((NT_TOTAL * P, D), np.float32)
    for c in range(NCORES):
        for j in range(TPC):
            t = int(orders[c, j])
            full[t * P:(t + 1) * P] = out_cores[c][:, j * P:(j + 1) * P].T
    return np.ascontiguousarray(full[:N])


# ------------------------------------------------------------- bass program
def build_program(B, Bcum, NBLK, EPAD, use_b1v, tpc=TPC, npc=None):
    npc = npc if npc is not None else tpc * P
    nc = bass.Bass("TRN2", target_bir_lowering=False, debug=False)
    XA_d = nc.declare_dram_parameter("XA", [128, EPAD], BF16, isOutput=False)
    XB_d = nc.declare_dram_parameter("XB", [33, EPAD], BF16, isOutput=False)
    DL_d = nc.declare_dram_parameter("DL", [128, NBLK], F32, isOutput=False)
    wnames = ["W0kA", "W0kB", "W0vA", "W0vB", "AQ8", "MW1v", "b1v_rep",
              "oW0", "ob0", "MoW1", "ob1"]
    wshapes = {"W0kA": [128, 128], "W0kB": [33, 128], "W0vA": [128, 128],
               "W0vB": [33, 128], "AQ8": [128, 8], "MW1v": [128, 128],
               "b1v_rep": [128, 128], "oW0": [128, 128], "ob0": [128, 1],
               "MoW1": [128, 128], "ob1": [128, 1]}
    wdt = {"b1v_rep": F32, "ob0": F32, "ob1": F32}
    w_d = {n: nc.declare_dram_parameter(n, wshapes[n], wdt.get(n, BF16),
                                        isOutput=False)
           for n in wnames}
    OUT_d = nc.declare_dram_parameter("OUT", [128, npc], F32, isOutput=True)

    SLAB = SLAB_BLOCKS * P
    nslabs = (NBLK + SLAB_BLOCKS - 1) // SLAB_BLOCKS

    with ExitStack() as ctx:
        tc = ctx.enter_context(tile.TileContext(nc))
        cpool = ctx.enter_context(tc.tile_pool(name="consts", bufs=1))
        xpool = ctx.enter_context(tc.tile_pool(name="x", bufs=3))
        hpool = ctx.enter_context(tc.tile_pool(name="h", bufs=2))
        ohpool = ctx.enter_context(tc.tile_pool(name="ohp", bufs=2))
        expool = ctx.enter_context(tc.tile_pool(name="exp", bufs=2))
        empool = ctx.enter_context(tc.tile_pool(name="em", bufs=6))
        npool = ctx.enter_context(tc.tile_pool(name="node", bufs=2))
        ps_l0 = ctx.enter_context(tc.tile_pool(name="psl0", bufs=1, space="PSUM"))
        ps_sc = ctx.enter_context(tc.tile_pool(name="pssc", bufs=2, space="PSUM"))
        ps_v = ctx.enter_context(tc.tile_pool(name="psv", bufs=2, space="PSUM"))
        ps_s = ctx.enter_context(tc.tile_pool(name="pss", bufs=2, space="PSUM"))

        # --- persistent constants ---
        w_sb = {}
        for n in wnames:
            t = cpool.tile(wshapes[n], wdt.get(n, BF16), name=f"w_{n}")
            nc.sync.dma_start(t[:], w_d[n][:])
            w_sb[n] = t
        dl_sb = cpool.tile([128, NBLK], F32, name="dl")
        nc.sync.dma_start(dl_sb[:], DL_d[:])
        iota_row_i = cpool.tile([128, 128], I32, name="iota_row_i")
        nc.gpsimd.iota(iota_row_i[:], pattern=[[1, 128]], base=0,
                       channel_multiplier=0)
        iota_row = cpool.tile([128, 128], F32, name="iota_row")
        nc.vector.tensor_copy(iota_row[:], iota_row_i[:])
        iota_p_i = cpool.tile([128, 1], I32, name="iota_p_i")
        nc.gpsimd.iota(iota_p_i[:], pattern=[[1, 1]], base=0,
                       channel_multiplier=1)
        iota_p = cpool.tile([128, 1], F32, name="iota_p")
        nc.vector.tensor_copy(iota_p[:], iota_p_i[:])
        ident_b = cpool.tile([128, 128], BF16, name="ident_b")
        nc.vector.tensor_scalar(ident_b[:], iota_row[:], iota_p[:], None,
                                op0=mybir.AluOpType.is_equal)

        headsel = np.arange(D) // DH  # feature -> head

        # --- main loop ---
        xa_t = xb_t = hk_t = hv_t = None
        slab_edges = 0
        j = 0  # current node tile
        S_ps = None
        for s in range(nslabs):
            b0 = s * SLAB_BLOCKS
            nblk_s = min(SLAB_BLOCKS, NBLK - b0)
            ne = nblk_s * P
            e0 = b0 * P
            xa_t = xpool.tile([128, SLAB], BF16, tag="xa", name=f"xa{s}")
            xb_t = xpool.tile([33, SLAB], BF16, tag="xb", name=f"xb{s}")
            nc.sync.dma_start(xa_t[:, :ne], XA_d[:, e0:e0 + ne])
            nc.sync.dma_start(xb_t[:, :ne], XB_d[:, e0:e0 + ne])
            hk_t = hpool.tile([128, SLAB], BF16, tag="hk", name=f"hk{s}")
            hv_t = hpool.tile([128, SLAB], BF16, tag="hv", name=f"hv{s}")
            # L0 in chunks of 512
            for c0 in range(0, ne, 512):
                cw = min(512, ne - c0)
                hk_ps = ps_l0.tile([128, 512], F32, tag="hkps", name=f"hkps{s}_{c0}")
                nc.tensor.matmul(hk_ps[:, :cw], w_sb["W0kA"][:],
                                 xa_t[:, c0:c0 + cw], start=True, stop=False)
                nc.tensor.matmul(hk_ps[:, :cw], w_sb["W0kB"][:],
                                 xb_t[:, c0:c0 + cw], start=False, stop=True)
                nc.scalar.activation(hk_t[:, c0:c0 + cw], hk_ps[:, :cw],
                                     mybir.ActivationFunctionType.Relu)
                hv_ps = ps_l0.tile([128, 512], F32, tag="hvps", name=f"hvps{s}_{c0}")
                nc.tensor.matmul(hv_ps[:, :cw], w_sb["W0vA"][:],
                                 xa_t[:, c0:c0 + cw], start=True, stop=False)
                nc.tensor.matmul(hv_ps[:, :cw], w_sb["W0vB"][:],
                                 xb_t[:, c0:c0 + cw], start=False, stop=True)
                nc.scalar.activation(hv_t[:, c0:c0 + cw], hv_ps[:, :cw],
                                     mybir.ActivationFunctionType.Relu)
            # one-hot for the whole slab (bf16): oh[p, 128*bb+n] = (dl==n)
            oh_t = ohpool.tile([128, SLAB_BLOCKS, P], BF16, tag="oh",
                               name=f"oh{s}")
            nc.vector.tensor_tensor(
                oh_t[:, :nblk_s, :],
                iota_row[:].unsqueeze(1).broadcast_to([128, nblk_s, P]),
                dl_sb[:, b0:b0 + nblk_s].unsqueeze(2).broadcast_to(
                    [128, nblk_s, P]),
                op=mybir.AluOpType.is_equal)
            # scores for the whole slab -> single exp
            sc_ps = ps_sc.tile([128, SLAB_BLOCKS, 8], F32, tag="sc",
                               name=f"sc{s}")
            for bb in range(nblk_s):
                nc.tensor.matmul(sc_ps[:, bb, :],
                                 hk_t[:, bb * P:(bb + 1) * P], w_sb["AQ8"][:],
                                 start=True, stop=True, skip_group_check=True)
            ex_t = expool.tile([128, SLAB_BLOCKS, 8], F32, tag="ex",
                               name=f"ex{s}")
            nc.scalar.activation(ex_t[:, :nblk_s, :], sc_ps[:, :nblk_s, :],
                                 mybir.ActivationFunctionType.Exp)
            # v matmuls per quad + batched exv mult / ex8 cast
            exvs_at = {}
            for q0 in range(0, nblk_s, 4):
                qn = min(4, nblk_s - q0)
                v_ps = ps_v.tile([128, 4, 128], F32, tag="vps",
                                 name=f"vp{s}_{q0}")
                for i in range(qn):
                    bb = q0 + i
                    nc.tensor.matmul(v_ps[:, i, :],
                                     hv_t[:, bb * P:(bb + 1) * P],
                                     w_sb["MW1v"][:], start=True, stop=True,
                                     skip_group_check=True)
                if use_b1v:
                    nc.vector.tensor_tensor(
                        v_ps[:, :qn, :],
                        v_ps[:, :qn, :],
                        w_sb["b1v_rep"][:].unsqueeze(1).broadcast_to(
                            [128, qn, 128]),
                        op=mybir.AluOpType.add)
                exvs = empool.tile([128, 4, 136], BF16, tag="exvs",
                                   name=f"exvs{s}_{q0}")
                nc.vector.tensor_copy(
                    exvs[:, :qn, 128:136],
                    ex_t[:, q0:q0 + qn, :])
                nc.vector.tensor_tensor(
                    exvs[:, :qn, 0:128].rearrange("p q (h r) -> p q h r", r=DH),
                    v_ps[:, :qn, :].rearrange("p q (h r) -> p q h r", r=DH),
                    ex_t[:, q0:q0 + qn, :].unsqueeze(3).broadcast_to(
                        [128, qn, 8, DH]),
                    op=mybir.AluOpType.mult)
                for i in range(qn):
                    exvs_at[q0 + i] = (exvs, i)
            # scatter + node epilogues
            for bb in range(nblk_s):
                b = b0 + bb
                if b == Bcum[j]:
                    S_ps = ps_s.tile([128, 144], F32, tag="S", name=f"S{j}")
                first = (b == Bcum[j])
                last = (b == Bcum[j + 1] - 1)
                exvs, i = exvs_at[bb]
                nc.tensor.matmul(S_ps[:, 0:136], oh_t[:, bb, :],
                                 exvs[:, i, :], start=first, stop=last,
                                 skip_group_check=True)

                if last:
                    # node-tile epilogue
                    s1 = npool.tile([128, 8], F32, tag="s1", name=f"s1_{j}")
                    nc.vector.tensor_scalar_max(s1[:], S_ps[:, 128:136], 1e-30)
                    r1 = npool.tile([128, 8], F32, tag="r1", name=f"r1_{j}")
                    nc.vector.reciprocal(r1[:], s1[:])
                    g = npool.tile([128, 128], BF16, tag="g", name=f"g{j}")
                    nc.vector.tensor_tensor(
                        g[:].rearrange("p (h r) -> p h r", r=DH),
                        S_ps[:, 0:128].rearrange("p (h r) -> p h r", r=DH),
                        r1[:].unsqueeze(2).broadcast_to([128, 8, DH]),
                        op=mybir.AluOpType.mult)
                    nc.vector.tensor_scalar_max(g[:], g[:], 0.0)
                    tp_ps = ps_v.tile([128, 128], BF16, tag="vps",
                                      name=f"tp{j}")
                    nc.tensor.transpose(tp_ps[:], g[:], ident_b[:])
                    gfm = npool.tile([128, 128], BF16, tag="gfm", name=f"gfm{j}")
                    nc.scalar.copy(gfm[:], tp_ps[:])
                    h0_ps = ps_v.tile([128, 128], F32, tag="vps", name=f"h0p{j}")
                    nc.tensor.matmul(h0_ps[:], w_sb["oW0"][:], gfm[:],
                                     start=True, stop=True)
                    h0 = npool.tile([128, 128], BF16, tag="h0", name=f"h0{j}")
                    nc.scalar.activation(h0[:], h0_ps[:],
                                         mybir.ActivationFunctionType.Relu,
                                         bias=w_sb["ob0"][:])
                    o2_ps = ps_v.tile([128, 128], F32, tag="vps", name=f"o2p{j}")
                    nc.tensor.matmul(o2_ps[:], w_sb["MoW1"][:], h0[:],
                                     start=True, stop=True)
                    ot = npool.tile([128, 128], F32, tag="ot", name=f"ot{j}")
                    nc.scalar.activation(ot[:], o2_ps[:],
                                         mybir.ActivationFunctionType.Relu,
                                         bias=w_sb["ob1"][:])
                    nc.sync.dma_start(OUT_d[:, j * P:(j + 1) * P], ot[:])
                    j += 1
    _split_excess_waits(nc)
    return nc


# ------------------------------------------------------------------ kernel
def kernel(**inputs):
    staged, weights, meta = _prep(inputs)
    nc = build_program(meta["B"], meta["Bcum"], meta["NBLK"], meta["EPAD"],
                       meta["use_b1v"])
    in_maps = []
    for c in range(NCORES):
        m = {"XA": staged["XA"][c], "XB": staged["XB"][c],
             "DL": staged["dstlocT"][c]}
        m.update(weights)
        in_maps.append(m)
    res = run_bass_kernel_spmd(nc, in_maps, list(range(NCORES)))
    out_cores = [res.results[c]["OUT"] for c in range(NCORES)]
    return _unshard(out_cores, meta["orders"])


# revision 23
# speedup vs baseline: 3.6627x; 1.0181x over previous
"""Trainium2 Bass kernel for nn_NeighborhoodAttention (GNN message passing).

Strategy (no collectives needed):
  - Host: sort edges by dst, pad nodes 50000->50176 = 392 tiles of 128; core c
    owns 49 contiguous node tiles and their (contiguous) edges. Per node tile,
    the edge list is padded to a multiple of 128 ("blocks"); within each core,
    tiles are processed in descending-edge-count order so the per-position
    block count B_j is shared across all 8 cores (single SPMD program).
  - Gathered endpoint features are staged feature-major ([feat, edge]); the
    per-edge MLPs run feature-major on TensorE with stationary weights.
    Residual layers and the q-dot are folded on the host:
      scores = ((I+kW1) @ qmask)^T h_k        (per-head additive consts cancel
      v      = (I+vW1)^T h_v + vb1             exactly in the segment softmax,
                                               incl. the max subtraction)
  - Edge-major tensors (scores_em, v_em) come from "em-mode" matmuls using the
    h-slab slice as the stationary operand - no transposes anywhere.
  - Segment softmax/scatter-add: per 128-edge block build onehot[e, n] on DVE
    (iota + is_equal vs dst_local), then accumulate in PSUM per node tile:
      S[:, 0:128] += onehot^T @ (v_em * exp(scores)_bcast),
      S[:, 128:136] += onehot^T @ exp(scores).
  - Node side: aggr = S / max(S1,eps) per head, relu, PE-transpose to
    feature-major, 2-layer output MLP, write feature-major; host untransposes.
"""
import os
import sys
from contextlib import ExitStack

import ml_dtypes
import numpy as np

sys.path.insert(0, "/opt/trn_rl_repo")

import concourse.bass as bass
import concourse.tile as tile
from concourse import mybir
from concourse.bass_utils import run_bass_kernel_spmd
from concourse.vector_clock import ScopedClock


def _patched_drain_and_barrier(self, tick_clock, wait_clock):
    # Workaround: walrus CoreV3 setupSyncWait rejects >couple sem-waits on a
    # CTRL-class (drain) instruction. Spread the tail-drain waits across
    # preceding sync-engine nops (1 wait each) and leave the drain clean.
    nc = self.nc
    nop0 = nc.sync.nop(hint="tile_drain_waits", nofuse=True)
    wait_clock.add_sem_waits(nop0.ins, ScopedClock({None: tick_clock.global_clock}))
    si = nop0.ins.sync_info
    waits = list(si.on_wait) if si is not None and si.on_wait else []
    if len(waits) > 1:
        si.on_wait = waits[:1]
        for w in waits[1:]:
            ni = nc.sync.nop(hint="tile_drain_waits", nofuse=True)
            nsi = ni.ins.sync_info
            if nsi is None:
                ni.ins.sync_info = mybir.SyncInfo(on_wait=[w], on_update=[])
            else:
                nsi.on_wait = [w]
    nc.sync.drain()
    nc.all_engine_barrier()
    popped = nc._tile_sem_poison_stack.pop()
    assert popped is self._sem_poison
    nc.clear_and_free_semaphores(list(self.sems.allocated().values()))
    nc.all_engine_barrier()


tile.TileContext._drain_and_barrier = _patched_drain_and_barrier


def _split_excess_waits(nc, max_waits=1):
    """Walrus CoreV3 setupSyncWait rejects instructions with more than one
    sem-wait. Hoist excess waits onto same-engine nops inserted just before
    the offending instruction (program order per engine is the bb order)."""
    f = nc.m.functions[0]
    offenders = {}  # name -> list of hoisted-nop Instructions
    created = set()
    for bb in f.blocks:
        for inst in bb.instructions:
            si = inst.sync_info
            if si is None or not si.on_wait or len(si.on_wait) <= max_waits:
                continue
            w = list(si.on_wait)
            nops = []
            for wt in w[:-max_waits]:
                bi = nc.engines[inst.engine].nop(nofuse=True)
                nsi = bi.ins.sync_info
                if nsi is None:
                    bi.ins.sync_info = mybir.SyncInfo(on_wait=[wt], on_update=[])
                else:
                    nsi.on_wait = [wt]
                nops.append(bi.ins)
                created.add(bi.ins.name)
            si.on_wait = w[-max_waits:]
            offenders[inst.name] = nops
    if not offenders:
        return
    for bb in f.blocks:
        insts = list(bb.instructions)
        out = []
        changed = False
        for inst in insts:
            if inst.name in created:
                changed = True
                continue
            if inst.name in offenders:
                out.extend(offenders[inst.name])
                changed = True
            out.append(inst)
        if changed:
            bb.instructions = out

# problem constants (hardcoded per contract)
N, E = 50000, 800000
SRCF, DSTF, EDGEF = 64, 64, 32
D, H, DH = 128, 8, 16
SCALE = 1.0 / np.sqrt(np.float32(DH))
NCORES = 8
P = 128
NT_TOTAL = 392
TPC = NT_TOTAL // NCORES        # 49 node tiles per core
NPC = TPC * P                   # 6272 nodes per core
SLAB_BLOCKS = 16                # edge DMA/L0 slab = 16 blocks = 2048 edges
F32 = mybir.dt.float32
I32 = mybir.dt.int32
BF16 = mybir.dt.bfloat16


# ----------------------------------------------------------------- host prep
def _prep(inputs):
    x_src = np.asarray(inputs["x_src"], np.float32)
    x_dst = np.asarray(inputs["x_dst"], np.float32)
    edge_attr = np.asarray(inputs["edge_attr"], np.float32)
    ei = np.asarray(inputs["edge_index"])
    src = ei[0].astype(np.int64)
    dst = ei[1].astype(np.int64)

    perm = np.argsort(dst, kind="stable")
    src_s, dst_s = src[perm], dst[perm]
    ea_s = edge_attr[perm]
    tile_counts = np.bincount(dst_s // P, minlength=NT_TOTAL)
    tile_starts = np.zeros(NT_TOTAL + 1, np.int64)
    np.cumsum(tile_counts, out=tile_starts[1:])

    orders = np.zeros((NCORES, TPC), np.int64)
    sorted_counts = np.zeros((NCORES, TPC), np.int64)
    for c in range(NCORES):
        tiles = np.arange(c * TPC, (c + 1) * TPC)
        o = np.argsort(-tile_counts[tiles], kind="stable")
        orders[c] = tiles[o]
        sorted_counts[c] = tile_counts[orders[c]]
    B = np.maximum(np.ceil(sorted_counts.max(axis=0) / P).astype(np.int64), 1)
    Bcum = np.zeros(TPC + 1, np.int64)
    np.cumsum(B, out=Bcum[1:])
    NBLK = int(B.sum())
    EPAD = NBLK * P

    slot = np.full((NCORES, EPAD), -1, np.int64)
    dstloc = np.full((NCORES, EPAD), -1, np.int64)
    for c in range(NCORES):
        for j in range(TPC):
            t = orders[c, j]
            s0, cnt = int(tile_starts[t]), int(tile_counts[t])
            pos = int(Bcum[j]) * P
            slot[c, pos:pos + cnt] = np.arange(s0, s0 + cnt)
            dstloc[c, pos:pos + cnt] = dst_s[s0:s0 + cnt] - t * P

    real = slot >= 0
    slot_c = np.where(real, slot, 0)
    XA = np.zeros((NCORES, 128, EPAD), ml_dtypes.bfloat16)
    XB = np.zeros((NCORES, 33, EPAD), ml_dtypes.bfloat16)
    for c in range(NCORES):
        r = real[c]
        XA[c, :64] = np.where(r, x_src[src_s[slot_c[c]]].T, 0)
        XA[c, 64:] = np.where(r, x_dst[dst_s[slot_c[c]]].T, 0)
        XB[c, :32] = np.where(r, ea_s[slot_c[c]].T, 0)
        XB[c, 32] = 1.0
    dstlocT = np.ascontiguousarray(
        dstloc.reshape(NCORES, NBLK, P).transpose(0, 2, 1)).astype(np.float32)

    kW0 = np.asarray(inputs["kW0"], np.float32)
    kb0 = np.asarray(inputs["kb0"], np.float32)
    kW1 = np.asarray(inputs["kW1"], np.float32)
    vW0 = np.asarray(inputs["vW0"], np.float32)
    vb0 = np.asarray(inputs["vb0"], np.float32)
    vW1 = np.asarray(inputs["vW1"], np.float32)
    vb1 = np.asarray(inputs["vb1"], np.float32)
    q = np.asarray(inputs["q"], np.float32)

    qmask = np.zeros((D, H), np.float32)
    for h in range(H):
        qmask[h * DH:(h + 1) * DH, h] = q[0, h * DH:(h + 1) * DH] * SCALE

    bf = ml_dtypes.bfloat16
    weights = dict(
        W0kA=np.ascontiguousarray(kW0[:128]).astype(bf),
        W0kB=np.concatenate([kW0[128:160], kb0[None, :]], 0).astype(bf),
        W0vA=np.ascontiguousarray(vW0[:128]).astype(bf),
        W0vB=np.concatenate([vW0[128:160], vb0[None, :]], 0).astype(bf),
        AQ8=((np.eye(D, dtype=np.float32) + kW1) @ qmask).astype(bf),
        MW1v=(np.eye(D, dtype=np.float32) + vW1).astype(bf),
        b1v_rep=np.tile(vb1[None, :], (P, 1)),
        oW0=np.asarray(inputs["oW0"], np.float32).astype(bf),
        ob0=np.asarray(inputs["ob0"], np.float32).reshape(P, 1),
        MoW1=(np.eye(D, dtype=np.float32)
              + np.asarray(inputs["oW1"], np.float32)).astype(bf),
        ob1=np.asarray(inputs["ob1"], np.float32).reshape(P, 1),
    )
    use_b1v = bool(np.any(weights["b1v_rep"]))
    meta = dict(B=B, Bcum=Bcum, NBLK=NBLK, EPAD=EPAD, orders=orders,
                use_b1v=use_b1v)
    staged = dict(XA=XA, XB=XB, dstlocT=dstlocT)
    return staged, weights, meta


def _unshard(out_cores, orders):
    full = np.zeros((NT_TOTAL * P, D), np.float32)
    for c in range(NCORES):
        for j in range(TPC):
            t = int(orders[c, j])
            full[t * P:(t + 1) * P] = out_cores[c][:, j * P:(j + 1) * P].T
    return np.ascontiguousarray(full[:N])


# ------------------------------------------------------------- bass program
def build_program(B, Bcum, NBLK, EPAD, use_b1v, tpc=TPC, npc=None):
    npc = npc if npc is not None else tpc * P
    nc = bass.Bass("TRN2", target_bir_lowering=False, debug=False)
    XA_d = nc.declare_dram_parameter("XA", [128, EPAD], BF16, isOutput=False)
    XB_d = nc.declare_dram_parameter("XB", [33, EPAD], BF16, isOutput=False)
    DL_d = nc.declare_dram_parameter("DL", [128, NBLK], F32, isOutput=False)
    wnames = ["W0kA", "W0kB", "W0vA", "W0vB", "AQ8", "MW1v", "b1v_rep",
              "oW0", "ob0", "MoW1", "ob1"]
    wshapes = {"W0kA": [128, 128], "W0kB": [33, 128], "W0vA": [128, 128],
               "W0vB": [33, 128], "AQ8": [128, 8], "MW1v": [128, 128],
               "b1v_rep": [128, 128], "oW0": [128, 128], "ob0": [128, 1],
               "MoW1": [128, 128], "ob1": [128, 1]}
    wdt = {"b1v_rep": F32, "ob0": F32, "ob1": F32}
    w_d = {n: nc.declare_dram_parameter(n, wshapes[n], wdt.get(n, BF16),
                                        isOutput=False)
           for n in wnames}
    OUT_d = nc.declare_dram_parameter("OUT", [128, npc], F32, isOutput=True)

    SLAB = SLAB_BLOCKS * P
    nslabs = (NBLK + SLAB_BLOCKS - 1) // SLAB_BLOCKS

    with ExitStack() as ctx:
        tc = ctx.enter_context(tile.TileContext(nc))
        cpool = ctx.enter_context(tc.tile_pool(name="consts", bufs=1))
        xpool = ctx.enter_context(tc.tile_pool(name="x", bufs=3))
        hpool = ctx.enter_context(tc.tile_pool(name="h", bufs=2))
        ohpool = ctx.enter_context(tc.tile_pool(name="ohp", bufs=3))
        expool = ctx.enter_context(tc.tile_pool(name="exp", bufs=3))
        empool = ctx.enter_context(tc.tile_pool(name="em", bufs=8))
        npool = ctx.enter_context(tc.tile_pool(name="node", bufs=2))
        ps_l0 = ctx.enter_context(tc.tile_pool(name="psl0", bufs=1, space="PSUM"))
        ps_sc = ctx.enter_context(tc.tile_pool(name="pssc", bufs=2, space="PSUM"))
        ps_v = ctx.enter_context(tc.tile_pool(name="psv", bufs=2, space="PSUM"))
        ps_s = ctx.enter_context(tc.tile_pool(name="pss", bufs=2, space="PSUM"))

        # --- persistent constants ---
        w_sb = {}
        for n in wnames:
            if n == "W0vB":
                t = cpool.tile([97, 128], BF16, name=f"w_{n}")
                nc.sync.dma_start(t[64:97, :], w_d[n][:])
            else:
                t = cpool.tile(wshapes[n], wdt.get(n, BF16), name=f"w_{n}")
                nc.sync.dma_start(t[:], w_d[n][:])
            w_sb[n] = t
        dl_sb = cpool.tile([128, NBLK], F32, name="dl")
        nc.sync.dma_start(dl_sb[:], DL_d[:])
        iota_row_i = cpool.tile([128, 128], I32, name="iota_row_i")
        nc.gpsimd.iota(iota_row_i[:], pattern=[[1, 128]], base=0,
                       channel_multiplier=0)
        iota_row = cpool.tile([128, 128], F32, name="iota_row")
        nc.vector.tensor_copy(iota_row[:], iota_row_i[:])
        iota_p_i = cpool.tile([128, 1], I32, name="iota_p_i")
        nc.gpsimd.iota(iota_p_i[:], pattern=[[1, 1]], base=0,
                       channel_multiplier=1)
        iota_p = cpool.tile([128, 1], F32, name="iota_p")
        nc.vector.tensor_copy(iota_p[:], iota_p_i[:])
        ident_b = cpool.tile([128, 128], BF16, name="ident_b")
        nc.vector.tensor_scalar(ident_b[:], iota_row[:], iota_p[:], None,
                                op0=mybir.AluOpType.is_equal)

        headsel = np.arange(D) // DH  # feature -> head

        # --- main loop ---
        xa_t = xb_t = hk_t = hv_t = None
        slab_edges = 0
        j = 0  # current node tile
        S_ps = None
        for s in range(nslabs):
            b0 = s * SLAB_BLOCKS
            nblk_s = min(SLAB_BLOCKS, NBLK - b0)
            ne = nblk_s * P
            e0 = b0 * P
            xa_t = xpool.tile([128, SLAB], BF16, tag="xa", name=f"xa{s}")
            xb_t = xpool.tile([97, SLAB], BF16, tag="xb", name=f"xb{s}")
            nc.sync.dma_start(xa_t[:, :ne], XA_d[:, e0:e0 + ne])
            nc.sync.dma_start(xb_t[0:33, :ne], XB_d[:, e0:e0 + ne])
            nc.sync.dma_start(xb_t[64:97, :ne], XB_d[:, e0:e0 + ne])
            hk_t = hpool.tile([128, SLAB], BF16, tag="hk", name=f"hk{s}")
            hv_t = hpool.tile([128, SLAB], BF16, tag="hv", name=f"hv{s}")
            # L0 in chunks of 512
            for c0 in range(0, ne, 512):
                cw = min(512, ne - c0)
                hk_ps = ps_l0.tile([128, 512], F32, tag="hkps", name=f"hkps{s}_{c0}")
                nc.tensor.matmul(hk_ps[:, :cw], w_sb["W0kA"][:],
                                 xa_t[:, c0:c0 + cw], start=True, stop=False)
                nc.tensor.matmul(hk_ps[:, :cw], w_sb["W0kB"][:, :],
                                 xb_t[0:33, c0:c0 + cw], start=False, stop=True)
                nc.scalar.activation(hk_t[:, c0:c0 + cw], hk_ps[:, :cw],
                                     mybir.ActivationFunctionType.Relu)
                hv_ps = ps_l0.tile([128, 512], F32, tag="hvps", name=f"hvps{s}_{c0}")
                nc.tensor.matmul(hv_ps[:, :cw], w_sb["W0vA"][:],
                                 xa_t[:, c0:c0 + cw], start=True, stop=False)
                nc.tensor.matmul(hv_ps[:, :cw], w_sb["W0vB"][64:97, :],
                                 xb_t[64:97, c0:c0 + cw], start=False,
                                 stop=True)
                nc.scalar.activation(hv_t[:, c0:c0 + cw], hv_ps[:, :cw],
                                     mybir.ActivationFunctionType.Relu)
            # one-hot for the whole slab (bf16): oh[p, 128*bb+n] = (dl==n)
            oh_t = ohpool.tile([128, SLAB_BLOCKS, P], BF16, tag="oh",
                               name=f"oh{s}")
            nc.vector.tensor_tensor(
                oh_t[:, :nblk_s, :],
                iota_row[:].unsqueeze(1).broadcast_to([128, nblk_s, P]),
                dl_sb[:, b0:b0 + nblk_s].unsqueeze(2).broadcast_to(
                    [128, nblk_s, P]),
                op=mybir.AluOpType.is_equal)
            # scores for the whole slab -> single exp
            sc_ps = ps_sc.tile([128, SLAB_BLOCKS, 8], F32, tag="sc",
                               name=f"sc{s}")
            for bb in range(nblk_s):
                nc.tensor.matmul(sc_ps[:, bb, :],
                                 hk_t[:, bb * P:(bb + 1) * P], w_sb["AQ8"][:],
                                 start=True, stop=True, skip_group_check=True)
            ex_t = expool.tile([128, SLAB_BLOCKS, 8], F32, tag="ex",
                               name=f"ex{s}")
            nc.scalar.activation(ex_t[:, :nblk_s, :], sc_ps[:, :nblk_s, :],
                                 mybir.ActivationFunctionType.Exp)
            # v matmuls per quad + batched exv mult / ex8 cast
            exvs_at = {}
            for q0 in range(0, nblk_s, 4):
                qn = min(4, nblk_s - q0)
                v_ps = ps_v.tile([128, 4, 128], F32, tag="vps",
                                 name=f"vp{s}_{q0}")
                for i in range(qn):
                    bb = q0 + i
                    nc.tensor.matmul(v_ps[:, i, :],
                                     hv_t[:, bb * P:(bb + 1) * P],
                                     w_sb["MW1v"][:], start=True, stop=True,
                                     skip_group_check=True)
                if use_b1v:
                    nc.vector.tensor_tensor(
                        v_ps[:, :qn, :],
                        v_ps[:, :qn, :],
                        w_sb["b1v_rep"][:].unsqueeze(1).broadcast_to(
                            [128, qn, 128]),
                        op=mybir.AluOpType.add)
                exvs = empool.tile([128, 4, 136], BF16, tag="exvs",
                                   name=f"exvs{s}_{q0}")
                nc.vector.tensor_copy(
                    exvs[:, :qn, 128:136],
                    ex_t[:, q0:q0 + qn, :])
                nc.vector.tensor_tensor(
                    exvs[:, :qn, 0:128].rearrange("p q (h r) -> p q h r", r=DH),
                    v_ps[:, :qn, :].rearrange("p q (h r) -> p q h r", r=DH),
                    ex_t[:, q0:q0 + qn, :].unsqueeze(3).broadcast_to(
                        [128, qn, 8, DH]),
                    op=mybir.AluOpType.mult)
                for i in range(qn):
                    exvs_at[q0 + i] = (exvs, i)
            # scatter + node epilogues
            for bb in range(nblk_s):
                b = b0 + bb
                if b == Bcum[j]:
                    S_ps = ps_s.tile([128, 144], F32, tag="S", name=f"S{j}")
                first = (b == Bcum[j])
                last = (b == Bcum[j + 1] - 1)
                exvs, i = exvs_at[bb]
                nc.tensor.matmul(S_ps[:, 0:136], oh_t[:, bb, :],
                                 exvs[:, i, :], start=first, stop=last,
                                 skip_group_check=True)

                if last:
                    # node-tile epilogue
                    s1 = npool.tile([128, 8], F32, tag="s1", name=f"s1_{j}")
                    nc.vector.tensor_scalar_max(s1[:], S_ps[:, 128:136], 1e-30)
                    r1 = npool.tile([128, 8], F32, tag="r1", name=f"r1_{j}")
                    nc.vector.reciprocal(r1[:], s1[:])
                    g = npool.tile([128, 128], BF16, tag="g", name=f"g{j}")
                    nc.vector.tensor_tensor(
                        g[:].rearrange("p (h r) -> p h r", r=DH),
                        S_ps[:, 0:128].rearrange("p (h r) -> p h r", r=DH),
                        r1[:].unsqueeze(2).broadcast_to([128, 8, DH]),
                        op=mybir.AluOpType.mult)
                    nc.vector.tensor_scalar_max(g[:], g[:], 0.0)
                    tp_ps = ps_v.tile([128, 128], BF16, tag="vps",
                                      name=f"tp{j}")
                    nc.tensor.transpose(tp_ps[:], g[:], ident_b[:])
                    gfm = npool.tile([128, 128], BF16, tag="gfm", name=f"gfm{j}")
                    nc.scalar.copy(gfm[:], tp_ps[:])
                    h0_ps = ps_v.tile([128, 128], F32, tag="vps", name=f"h0p{j}")
                    nc.tensor.matmul(h0_ps[:], w_sb["oW0"][:], gfm[:],
                                     start=True, stop=True)
                    h0 = npool.tile([128, 128], BF16, tag="h0", name=f"h0{j}")
                    nc.scalar.activation(h0[:], h0_ps[:],
                                         mybir.ActivationFunctionType.Relu,
                                         bias=w_sb["ob0"][:])
                    o2_ps = ps_v.tile([128, 128], F32, tag="vps", name=f"o2p{j}")
                    nc.tensor.matmul(o2_ps[:], w_sb["MoW1"][:], h0[:],
                                     start=True, stop=True)
                    ot = npool.tile([128, 128], F32, tag="ot", name=f"ot{j}")
                    nc.scalar.activation(ot[:], o2_ps[:],
                                         mybir.ActivationFunctionType.Relu,
                                         bias=w_sb["ob1"][:])
                    nc.sync.dma_start(OUT_d[:, j * P:(j + 1) * P], ot[:])
                    j += 1
    _split_excess_waits(nc)
    return nc


# ------------------------------------------------------------------ kernel
def kernel(**inputs):
    staged, weights, meta = _prep(inputs)
    nc = build_program(meta["B"], meta["Bcum"], meta["NBLK"], meta["EPAD"],
                       meta["use_b1v"])
    in_maps = []
    for c in range(NCORES):
        m = {"XA": staged["XA"][c], "XB": staged["XB"][c],
             "DL": staged["dstlocT"][c]}
        m.update(weights)
        in_maps.append(m)
    res = run_bass_kernel_spmd(nc, in_maps, list(range(NCORES)))
    out_cores = [res.results[c]["OUT"] for c in range(NCORES)]
    return _unshard(out_cores, meta["orders"])


# revision 24
# speedup vs baseline: 4.2225x; 1.1529x over previous
"""Trainium2 Bass kernel for nn_NeighborhoodAttention (GNN message passing).

Strategy (no collectives needed):
  - Host: sort edges by dst, pad nodes 50000->50176 = 392 tiles of 128; core c
    owns 49 contiguous node tiles and their (contiguous) edges. Per node tile,
    the edge list is padded to a multiple of 128 ("blocks"); within each core,
    tiles are processed in descending-edge-count order so the per-position
    block count B_j is shared across all 8 cores (single SPMD program).
  - Gathered endpoint features are staged feature-major ([feat, edge]); the
    per-edge MLPs run feature-major on TensorE with stationary weights.
    Residual layers and the q-dot are folded on the host:
      scores = ((I+kW1) @ qmask)^T h_k        (per-head additive consts cancel
      v      = (I+vW1)^T h_v + vb1             exactly in the segment softmax,
                                               incl. the max subtraction)
  - Edge-major tensors (scores_em, v_em) come from "em-mode" matmuls using the
    h-slab slice as the stationary operand - no transposes anywhere.
  - Segment softmax/scatter-add: per 128-edge block build onehot[e, n] on DVE
    (iota + is_equal vs dst_local), then accumulate in PSUM per node tile:
      S[:, 0:128] += onehot^T @ (v_em * exp(scores)_bcast),
      S[:, 128:136] += onehot^T @ exp(scores).
  - Node side: aggr = S / max(S1,eps) per head, relu, PE-transpose to
    feature-major, 2-layer output MLP, write feature-major; host untransposes.
"""
import os
import sys
from contextlib import ExitStack

import ml_dtypes
import numpy as np

sys.path.insert(0, "/opt/trn_rl_repo")

import concourse.bass as bass
import concourse.tile as tile
from concourse import mybir
from concourse.bass_utils import run_bass_kernel_spmd
from concourse.vector_clock import ScopedClock


def _patched_drain_and_barrier(self, tick_clock, wait_clock):
    # Workaround: walrus CoreV3 setupSyncWait rejects >couple sem-waits on a
    # CTRL-class (drain) instruction. Spread the tail-drain waits across
    # preceding sync-engine nops (1 wait each) and leave the drain clean.
    nc = self.nc
    nop0 = nc.sync.nop(hint="tile_drain_waits", nofuse=True)
    wait_clock.add_sem_waits(nop0.ins, ScopedClock({None: tick_clock.global_clock}))
    si = nop0.ins.sync_info
    waits = list(si.on_wait) if si is not None and si.on_wait else []
    if len(waits) > 1:
        si.on_wait = waits[:1]
        for w in waits[1:]:
            ni = nc.sync.nop(hint="tile_drain_waits", nofuse=True)
            nsi = ni.ins.sync_info
            if nsi is None:
                ni.ins.sync_info = mybir.SyncInfo(on_wait=[w], on_update=[])
            else:
                nsi.on_wait = [w]
    nc.sync.drain()
    nc.all_engine_barrier()
    popped = nc._tile_sem_poison_stack.pop()
    assert popped is self._sem_poison
    nc.clear_and_free_semaphores(list(self.sems.allocated().values()))
    nc.all_engine_barrier()


tile.TileContext._drain_and_barrier = _patched_drain_and_barrier


def _split_excess_waits(nc, max_waits=1):
    """Walrus CoreV3 setupSyncWait rejects instructions with more than one
    sem-wait. Hoist excess waits onto same-engine nops inserted just before
    the offending instruction (program order per engine is the bb order)."""
    f = nc.m.functions[0]
    offenders = {}  # name -> list of hoisted-nop Instructions
    created = set()
    for bb in f.blocks:
        for inst in bb.instructions:
            si = inst.sync_info
            if si is None or not si.on_wait or len(si.on_wait) <= max_waits:
                continue
            w = list(si.on_wait)
            nops = []
            for wt in w[:-max_waits]:
                bi = nc.engines[inst.engine].nop(nofuse=True)
                nsi = bi.ins.sync_info
                if nsi is None:
                    bi.ins.sync_info = mybir.SyncInfo(on_wait=[wt], on_update=[])
                else:
                    nsi.on_wait = [wt]
                nops.append(bi.ins)
                created.add(bi.ins.name)
            si.on_wait = w[-max_waits:]
            offenders[inst.name] = nops
    if not offenders:
        return
    for bb in f.blocks:
        insts = list(bb.instructions)
        out = []
        changed = False
        for inst in insts:
            if inst.name in created:
                changed = True
                continue
            if inst.name in offenders:
                out.extend(offenders[inst.name])
                changed = True
            out.append(inst)
        if changed:
            bb.instructions = out

# problem constants (hardcoded per contract)
N, E = 50000, 800000
SRCF, DSTF, EDGEF = 64, 64, 32
D, H, DH = 128, 8, 16
SCALE = 1.0 / np.sqrt(np.float32(DH))
NCORES = 8
P = 128
NT_TOTAL = 392
TPC = NT_TOTAL // NCORES        # 49 node tiles per core
NPC = TPC * P                   # 6272 nodes per core
SLAB_BLOCKS = 16                # edge DMA/L0 slab = 16 blocks = 2048 edges
F32 = mybir.dt.float32
I32 = mybir.dt.int32
BF16 = mybir.dt.bfloat16


# ----------------------------------------------------------------- host prep
def _prep(inputs):
    x_src = np.asarray(inputs["x_src"], np.float32)
    x_dst = np.asarray(inputs["x_dst"], np.float32)
    edge_attr = np.asarray(inputs["edge_attr"], np.float32)
    ei = np.asarray(inputs["edge_index"])
    src = ei[0].astype(np.int64)
    dst = ei[1].astype(np.int64)

    perm = np.argsort(dst, kind="stable")
    src_s, dst_s = src[perm], dst[perm]
    ea_s = edge_attr[perm]
    tile_counts = np.bincount(dst_s // P, minlength=NT_TOTAL)
    tile_starts = np.zeros(NT_TOTAL + 1, np.int64)
    np.cumsum(tile_counts, out=tile_starts[1:])

    orders = np.zeros((NCORES, TPC), np.int64)
    sorted_counts = np.zeros((NCORES, TPC), np.int64)
    for c in range(NCORES):
        tiles = np.arange(c * TPC, (c + 1) * TPC)
        o = np.argsort(-tile_counts[tiles], kind="stable")
        orders[c] = tiles[o]
        sorted_counts[c] = tile_counts[orders[c]]
    B = np.maximum(np.ceil(sorted_counts.max(axis=0) / P).astype(np.int64), 1)
    Bcum = np.zeros(TPC + 1, np.int64)
    np.cumsum(B, out=Bcum[1:])
    NBLK = int(B.sum())
    EPAD = NBLK * P

    slot = np.full((NCORES, EPAD), -1, np.int64)
    dstloc = np.full((NCORES, EPAD), -1, np.int64)
    for c in range(NCORES):
        for j in range(TPC):
            t = orders[c, j]
            s0, cnt = int(tile_starts[t]), int(tile_counts[t])
            pos = int(Bcum[j]) * P
            slot[c, pos:pos + cnt] = np.arange(s0, s0 + cnt)
            dstloc[c, pos:pos + cnt] = dst_s[s0:s0 + cnt] - t * P

    real = slot >= 0
    slot_c = np.where(real, slot, 0)
    XA = np.zeros((NCORES, 128, EPAD), ml_dtypes.bfloat16)
    XB = np.zeros((NCORES, 33, EPAD), ml_dtypes.bfloat16)
    for c in range(NCORES):
        r = real[c]
        XA[c, :64] = np.where(r, x_src[src_s[slot_c[c]]].T, 0)
        XA[c, 64:] = np.where(r, x_dst[dst_s[slot_c[c]]].T, 0)
        XB[c, :32] = np.where(r, ea_s[slot_c[c]].T, 0)
        XB[c, 32] = 1.0
    dstlocT = np.ascontiguousarray(
        dstloc.reshape(NCORES, NBLK, P).transpose(0, 2, 1)).astype(np.float32)

    kW0 = np.asarray(inputs["kW0"], np.float32)
    kb0 = np.asarray(inputs["kb0"], np.float32)
    kW1 = np.asarray(inputs["kW1"], np.float32)
    vW0 = np.asarray(inputs["vW0"], np.float32)
    vb0 = np.asarray(inputs["vb0"], np.float32)
    vW1 = np.asarray(inputs["vW1"], np.float32)
    vb1 = np.asarray(inputs["vb1"], np.float32)
    q = np.asarray(inputs["q"], np.float32)

    qmask = np.zeros((D, H), np.float32)
    for h in range(H):
        qmask[h * DH:(h + 1) * DH, h] = q[0, h * DH:(h + 1) * DH] * SCALE

    bf = ml_dtypes.bfloat16
    weights = dict(
        W0kA=np.ascontiguousarray(kW0[:128]).astype(bf),
        W0kB=np.concatenate([kW0[128:160], kb0[None, :]], 0).astype(bf),
        W0vA=np.ascontiguousarray(vW0[:128]).astype(bf),
        W0vB=np.concatenate([vW0[128:160], vb0[None, :]], 0).astype(bf),
        AQ8=((np.eye(D, dtype=np.float32) + kW1) @ qmask).astype(bf),
        MW1v=(np.eye(D, dtype=np.float32) + vW1).astype(bf),
        b1v_rep=np.tile(vb1[None, :], (P, 1)),
        oW0=np.asarray(inputs["oW0"], np.float32).astype(bf),
        ob0=np.asarray(inputs["ob0"], np.float32).reshape(P, 1),
        MoW1=(np.eye(D, dtype=np.float32)
              + np.asarray(inputs["oW1"], np.float32)).astype(bf),
        ob1=np.asarray(inputs["ob1"], np.float32).reshape(P, 1),
    )
    use_b1v = bool(np.any(weights["b1v_rep"]))
    meta = dict(B=B, Bcum=Bcum, NBLK=NBLK, EPAD=EPAD, orders=orders,
                use_b1v=use_b1v)
    staged = dict(XA=XA, XB=XB, dstlocT=dstlocT)
    return staged, weights, meta


def _unshard(out_cores, orders):
    full = np.zeros((NT_TOTAL * P, D), np.float32)
    for c in range(NCORES):
        for j in range(TPC):
            t = int(orders[c, j])
            full[t * P:(t + 1) * P] = out_cores[c][:, j * P:(j + 1) * P].T
    return np.ascontiguousarray(full[:N])


# ------------------------------------------------------------- bass program
def build_program(B, Bcum, NBLK, EPAD, use_b1v, tpc=TPC, npc=None):
    npc = npc if npc is not None else tpc * P
    nc = bass.Bass("TRN2", target_bir_lowering=False, debug=False)
    XA_d = nc.declare_dram_parameter("XA", [128, EPAD], BF16, isOutput=False)
    XB_d = nc.declare_dram_parameter("XB", [33, EPAD], BF16, isOutput=False)
    DL_d = nc.declare_dram_parameter("DL", [128, NBLK], F32, isOutput=False)
    wnames = ["W0kA", "W0kB", "W0vA", "W0vB", "AQ8", "MW1v", "b1v_rep",
              "oW0", "ob0", "MoW1", "ob1"]
    wshapes = {"W0kA": [128, 128], "W0kB": [33, 128], "W0vA": [128, 128],
               "W0vB": [33, 128], "AQ8": [128, 8], "MW1v": [128, 128],
               "b1v_rep": [128, 128], "oW0": [128, 128], "ob0": [128, 1],
               "MoW1": [128, 128], "ob1": [128, 1]}
    wdt = {"b1v_rep": F32, "ob0": F32, "ob1": F32}
    w_d = {n: nc.declare_dram_parameter(n, wshapes[n], wdt.get(n, BF16),
                                        isOutput=False)
           for n in wnames}
    OUT_d = nc.declare_dram_parameter("OUT", [128, npc], F32, isOutput=True)

    SLAB = SLAB_BLOCKS * P
    nslabs = (NBLK + SLAB_BLOCKS - 1) // SLAB_BLOCKS

    with ExitStack() as ctx:
        tc = ctx.enter_context(tile.TileContext(nc))
        cpool = ctx.enter_context(tc.tile_pool(name="consts", bufs=1))
        xpool = ctx.enter_context(tc.tile_pool(name="x", bufs=3))
        hpool = ctx.enter_context(tc.tile_pool(name="h", bufs=3))
        ohpool = ctx.enter_context(tc.tile_pool(name="ohp", bufs=3))
        expool = ctx.enter_context(tc.tile_pool(name="exp", bufs=3))
        empool = ctx.enter_context(tc.tile_pool(name="em", bufs=8))
        npool = ctx.enter_context(tc.tile_pool(name="node", bufs=2))
        ps_l0 = ctx.enter_context(tc.tile_pool(name="psl0", bufs=1, space="PSUM"))
        ps_sc = ctx.enter_context(tc.tile_pool(name="pssc", bufs=1, space="PSUM"))
        ps_v = ctx.enter_context(tc.tile_pool(name="psv", bufs=3, space="PSUM"))
        ps_s = ctx.enter_context(tc.tile_pool(name="pss", bufs=2, space="PSUM"))

        # --- persistent constants ---
        w_sb = {}
        for n in wnames:
            if n == "W0vB":
                t = cpool.tile([97, 128], BF16, name=f"w_{n}")
                nc.sync.dma_start(t[64:97, :], w_d[n][:])
            else:
                t = cpool.tile(wshapes[n], wdt.get(n, BF16), name=f"w_{n}")
                nc.sync.dma_start(t[:], w_d[n][:])
            w_sb[n] = t
        dl_sb = cpool.tile([128, NBLK], F32, name="dl")
        nc.sync.dma_start(dl_sb[:], DL_d[:])
        iota_row_i = cpool.tile([128, 128], I32, name="iota_row_i")
        nc.gpsimd.iota(iota_row_i[:], pattern=[[1, 128]], base=0,
                       channel_multiplier=0)
        iota_row = cpool.tile([128, 128], F32, name="iota_row")
        nc.vector.tensor_copy(iota_row[:], iota_row_i[:])
        iota_p_i = cpool.tile([128, 1], I32, name="iota_p_i")
        nc.gpsimd.iota(iota_p_i[:], pattern=[[1, 1]], base=0,
                       channel_multiplier=1)
        iota_p = cpool.tile([128, 1], F32, name="iota_p")
        nc.vector.tensor_copy(iota_p[:], iota_p_i[:])
        ident_b = cpool.tile([128, 128], BF16, name="ident_b")
        nc.vector.tensor_scalar(ident_b[:], iota_row[:], iota_p[:], None,
                                op0=mybir.AluOpType.is_equal)

        headsel = np.arange(D) // DH  # feature -> head

        # --- main loop ---
        xa_t = xb_t = hk_t = hv_t = None
        slab_edges = 0
        j = 0  # current node tile
        S_ps = None
        for s in range(nslabs):
            b0 = s * SLAB_BLOCKS
            nblk_s = min(SLAB_BLOCKS, NBLK - b0)
            ne = nblk_s * P
            e0 = b0 * P
            xa_t = xpool.tile([128, SLAB], BF16, tag="xa", name=f"xa{s}")
            xb_t = xpool.tile([97, SLAB], BF16, tag="xb", name=f"xb{s}")
            nc.sync.dma_start(xa_t[:, :ne], XA_d[:, e0:e0 + ne])
            nc.sync.dma_start(xb_t[0:33, :ne], XB_d[:, e0:e0 + ne])
            nc.sync.dma_start(xb_t[64:97, :ne], XB_d[:, e0:e0 + ne])
            hk_t = hpool.tile([128, SLAB], BF16, tag="hk", name=f"hk{s}")
            hv_t = hpool.tile([128, SLAB], BF16, tag="hv", name=f"hv{s}")
            # L0 in chunks of 512
            for c0 in range(0, ne, 512):
                cw = min(512, ne - c0)
                hk_ps = ps_l0.tile([128, 512], F32, tag="hkps", name=f"hkps{s}_{c0}")
                nc.tensor.matmul(hk_ps[:, :cw], w_sb["W0kA"][:],
                                 xa_t[:, c0:c0 + cw], start=True, stop=False)
                nc.tensor.matmul(hk_ps[:, :cw], w_sb["W0kB"][:, :],
                                 xb_t[0:33, c0:c0 + cw], start=False, stop=True)
                nc.scalar.activation(hk_t[:, c0:c0 + cw], hk_ps[:, :cw],
                                     mybir.ActivationFunctionType.Relu)
                hv_ps = ps_l0.tile([128, 512], F32, tag="hvps", name=f"hvps{s}_{c0}")
                nc.tensor.matmul(hv_ps[:, :cw], w_sb["W0vA"][:],
                                 xa_t[:, c0:c0 + cw], start=True, stop=False)
                nc.tensor.matmul(hv_ps[:, :cw], w_sb["W0vB"][64:97, :],
                                 xb_t[64:97, c0:c0 + cw], start=False,
                                 stop=True)
                nc.scalar.activation(hv_t[:, c0:c0 + cw], hv_ps[:, :cw],
                                     mybir.ActivationFunctionType.Relu)
            oh_t = ohpool.tile([128, SLAB_BLOCKS, P], BF16, tag="oh",
                               name=f"oh{s}")
            # scores for the whole slab -> single exp
            sc_ps = ps_sc.tile([128, SLAB_BLOCKS, 8], F32, tag="sc",
                               name=f"sc{s}")
            for bb in range(nblk_s):
                nc.tensor.matmul(sc_ps[:, bb, :],
                                 hk_t[:, bb * P:(bb + 1) * P], w_sb["AQ8"][:],
                                 start=True, stop=True, skip_group_check=True)
            ex_t = expool.tile([128, SLAB_BLOCKS, 8], F32, tag="ex",
                               name=f"ex{s}")
            nc.scalar.activation(ex_t[:, :nblk_s, :], sc_ps[:, :nblk_s, :],
                                 mybir.ActivationFunctionType.Exp)
            # v matmuls per quad + batched exv mult / ex8 cast
            exvs_at = {}
            for q0 in range(0, nblk_s, 4):
                qn = min(4, nblk_s - q0)
                v_ps = ps_v.tile([128, 4, 128], F32, tag="vps",
                                 name=f"vp{s}_{q0}")
                for i in range(qn):
                    bb = q0 + i
                    nc.tensor.matmul(v_ps[:, i, :],
                                     hv_t[:, bb * P:(bb + 1) * P],
                                     w_sb["MW1v"][:], start=True, stop=True,
                                     skip_group_check=True)
                if use_b1v:
                    nc.vector.tensor_tensor(
                        v_ps[:, :qn, :],
                        v_ps[:, :qn, :],
                        w_sb["b1v_rep"][:].unsqueeze(1).broadcast_to(
                            [128, qn, 128]),
                        op=mybir.AluOpType.add)
                exvs = empool.tile([128, 4, 136], BF16, tag="exvs",
                                   name=f"exvs{s}_{q0}")
                nc.vector.tensor_copy(
                    exvs[:, :qn, 128:136],
                    ex_t[:, q0:q0 + qn, :])
                nc.vector.tensor_tensor(
                    exvs[:, :qn, 0:128].rearrange("p q (h r) -> p q h r", r=DH),
                    v_ps[:, :qn, :].rearrange("p q (h r) -> p q h r", r=DH),
                    ex_t[:, q0:q0 + qn, :].unsqueeze(3).broadcast_to(
                        [128, qn, 8, DH]),
                    op=mybir.AluOpType.mult)
                nc.vector.tensor_tensor(
                    oh_t[:, q0:q0 + qn, :],
                    iota_row[:].unsqueeze(1).broadcast_to([128, qn, P]),
                    dl_sb[:, b0 + q0:b0 + q0 + qn].unsqueeze(2).broadcast_to(
                        [128, qn, P]),
                    op=mybir.AluOpType.is_equal)
                for i in range(qn):
                    exvs_at[q0 + i] = (exvs, i)
            # scatter + node epilogues
            for bb in range(nblk_s):
                b = b0 + bb
                if b == Bcum[j]:
                    S_ps = ps_s.tile([128, 144], F32, tag="S", name=f"S{j}")
                first = (b == Bcum[j])
                last = (b == Bcum[j + 1] - 1)
                exvs, i = exvs_at[bb]
                nc.tensor.matmul(S_ps[:, 0:136], oh_t[:, bb, :],
                                 exvs[:, i, :], start=first, stop=last,
                                 skip_group_check=True)

                if last:
                    # node-tile epilogue
                    s1 = npool.tile([128, 8], F32, tag="s1", name=f"s1_{j}")
                    nc.vector.tensor_scalar_max(s1[:], S_ps[:, 128:136], 1e-30)
                    r1 = npool.tile([128, 8], F32, tag="r1", name=f"r1_{j}")
                    nc.vector.reciprocal(r1[:], s1[:])
                    g = npool.tile([128, 128], BF16, tag="g", name=f"g{j}")
                    nc.vector.tensor_tensor(
                        g[:].rearrange("p (h r) -> p h r", r=DH),
                        S_ps[:, 0:128].rearrange("p (h r) -> p h r", r=DH),
                        r1[:].unsqueeze(2).broadcast_to([128, 8, DH]),
                        op=mybir.AluOpType.mult)
                    nc.vector.tensor_scalar_max(g[:], g[:], 0.0)
                    tp_ps = ps_v.tile([128, 128], BF16, tag="vps",
                                      name=f"tp{j}")
                    nc.tensor.transpose(tp_ps[:], g[:], ident_b[:])
                    gfm = npool.tile([128, 128], BF16, tag="gfm", name=f"gfm{j}")
                    nc.scalar.copy(gfm[:], tp_ps[:])
                    h0_ps = ps_v.tile([128, 128], F32, tag="vps", name=f"h0p{j}")
                    nc.tensor.matmul(h0_ps[:], w_sb["oW0"][:], gfm[:],
                                     start=True, stop=True)
                    h0 = npool.tile([128, 128], BF16, tag="h0", name=f"h0{j}")
                    nc.scalar.activation(h0[:], h0_ps[:],
                                         mybir.ActivationFunctionType.Relu,
                                         bias=w_sb["ob0"][:])
                    o2_ps = ps_v.tile([128, 128], F32, tag="vps", name=f"o2p{j}")
                    nc.tensor.matmul(o2_ps[:], w_sb["MoW1"][:], h0[:],
                                     start=True, stop=True)
                    ot = npool.tile([128, 128], F32, tag="ot", name=f"ot{j}")
                    nc.scalar.activation(ot[:], o2_ps[:],
                                         mybir.ActivationFunctionType.Relu,
                                         bias=w_sb["ob1"][:])
                    nc.sync.dma_start(OUT_d[:, j * P:(j + 1) * P], ot[:])
                    j += 1
    _split_excess_waits(nc)
    return nc


# ------------------------------------------------------------------ kernel
def kernel(**inputs):
    staged, weights, meta = _prep(inputs)
    nc = build_program(meta["B"], meta["Bcum"], meta["NBLK"], meta["EPAD"],
                       meta["use_b1v"])
    in_maps = []
    for c in range(NCORES):
        m = {"XA": staged["XA"][c], "XB": staged["XB"][c],
             "DL": staged["dstlocT"][c]}
        m.update(weights)
        in_maps.append(m)
    res = run_bass_kernel_spmd(nc, in_maps, list(range(NCORES)))
    out_cores = [res.results[c]["OUT"] for c in range(NCORES)]
    return _unshard(out_cores, meta["orders"])
